# revision 1
# baseline (speedup 1.0000x reference)
"""MCR loss kernel for Trainium2 (8 NeuronCores).

Strategy:
  - Shard batch T=16 -> 2 timesteps per core (data parallel, no collectives).
  - Per core, on device: 8x8 avg-pool (as sum; the 1/64 is folded into the
    conv weights) via vector-engine strided reduces; reflect-pad + 3x3 conv
    as 3 PE matmuls with K=(dy,ic)=96; LeakyReLU(0.2); Gram G_t = V_t V_t^T
    via PE transpose + matmul, contraction over the 576 pixels.
  - Host: matrix determinant lemma
        logdet(I_576 + a V^T V) = logdet(I_96 + a V V^T)
    so only the [2,96,96] Grams leave the device; float64 Cholesky logdets
    (16 x 96x96 + 48 x 32x32, ~5 MFLOP total) finish the scalar loss.
"""

import numpy as np

_STATE = {}

# -------- fixed problem geometry (hardcoded per harness contract) --------
B, CCH, H, W = 16, 32, 192, 192
NCORES = 8
TPC = B // NCORES          # timesteps per core = 2
OUT = 24                   # pooled spatial size
PIX = OUT * OUT            # 576
M = 96                     # feature rows (3 maps x 32 channels)
ALPHA_E = 6.0              # 576 / (96 * eps)
ALPHA_C = 18.0             # 576 / (32 * eps)


DEBUG_TAPS = False


def _build_nc():
    import concourse.bass as bass
    import concourse.tile as tile
    from concourse import bacc, mybir

    DT = mybir.dt.float32
    nc = bacc.Bacc(
        "TRN2", target_bir_lowering=False, debug=False, num_devices=NCORES
    )
    if DEBUG_TAPS:
        pooled_out = nc.declare_dram_parameter(
            "pooled_out", [192, PIX], DT, isOutput=True
        )
        v_out = nc.declare_dram_parameter("v_out", [96, TPC * PIX], DT, isOutput=True)

    # x[g] for g = t*3+m : feature-map plane stacks, host-reordered
    x = nc.declare_dram_parameter("x", [TPC * 3, CCH, H, W], DT, isOutput=False)
    wt = nc.declare_dram_parameter("wt", [3, 3, 96, 32], DT, isOutput=False)
    ident = nc.declare_dram_parameter("ident", [128, 128], DT, isOutput=False)
    g_out = nc.declare_dram_parameter("g_out", [TPC, M, M], DT, isOutput=True)

    # group g = t*3+m; pass A = groups 0..3 (128 partitions), B = 4..5
    groups = [(t, m) for t in range(TPC) for m in range(3)]
    ACT = mybir.ActivationFunctionType

    with tile.TileContext(nc) as tc:
        with (
            tc.tile_pool(name="persist", bufs=1) as persist,
            tc.tile_pool(name="slabA", bufs=2) as slabA_pool,
            tc.tile_pool(name="slabB", bufs=2) as slabB_pool,
            tc.tile_pool(name="convtmp", bufs=2) as convtmp,
            tc.tile_pool(name="vtpool", bufs=3) as vtpool,
            tc.tile_pool(name="psum", bufs=2, space="PSUM") as psum_pool,
            tc.tile_pool(name="psumg", bufs=2, space="PSUM") as psumg_pool,
        ):
            wt_sb = persist.tile([96, 288], DT, tag="wt")
            nc.gpsimd.dma_start(
                out=wt_sb[:].rearrange("p (m x c) -> p m x c", m=3, x=3),
                in_=wt.ap().rearrange("m x p c -> p m x c"),
            )
            id_sb = persist.tile([128, 128], DT, tag="ident")
            nc.gpsimd.dma_start(out=id_sb[:], in_=ident.ap())

            pooledA = persist.tile([128, PIX], DT, tag="pooledA")
            pooledB = persist.tile([64, PIX], DT, tag="pooledB")
            v_sb = persist.tile([96, TPC * PIX], DT, tag="v")
            g_sb = persist.tile([96, TPC * 96], DT, tag="g")

            # ---- pooling: 4 quarter-slabs per pass; partition=(grp,ch) ----
            # quarter q covers input rows 48q..48q+47 = output rows 6q..6q+5
            for part, pool_p, pooled in (
                ("A", slabA_pool, pooledA),
                ("B", slabB_pool, pooledB),
            ):
                npart = 128 if part == "A" else 64
                glo = 0 if part == "A" else 4
                for q in range(4):
                    slab = pool_p.tile([npart, 48 * W], DT, tag=f"slab{part}")
                    rows = slice(48 * q, 48 * (q + 1))
                    nc.sync.dma_start(
                        out=slab[:],
                        in_=x.ap()[glo : glo + npart // 32, :, rows, :].rearrange(
                            "g c h w -> (g c) h w"
                        ),
                    )
                    # two-stage pool: contiguous-innermost first (DVE fast
                    # path), then the strided row reduction on the 8x-smaller
                    # intermediate
                    wsum = pool_p.tile([npart, 6 * 8 * 24], DT, tag=f"wsum{part}")
                    nc.vector.tensor_reduce(
                        out=wsum[:],
                        in_=slab[:].rearrange("p (g w) -> p g w", w=8),
                        axis=mybir.AxisListType.X,
                        op=mybir.AluOpType.add,
                    )
                    nc.vector.tensor_reduce(
                        out=pooled[:, q * 144 : (q + 1) * 144],
                        in_=wsum[:].rearrange("p (y r x) -> p y x r", r=8, x=24),
                        axis=mybir.AxisListType.X,
                        op=mybir.AluOpType.add,
                    )

            # ---- conv per group: reflect pad, 3x replicate, 3 matmuls ----
            for gi, (t, m) in enumerate(groups):
                pooled = pooledA if gi < 4 else pooledB
                po = (gi % 4) * 32 if gi < 4 else (gi - 4) * 32
                psrc = pooled[po : po + 32, :]
                p3 = psrc.rearrange("p (y x) -> p y x", y=OUT)

                xpad = convtmp.tile([32, 26 * 26], DT, tag="xpad")
                x3 = xpad[:].rearrange("p (y x) -> p y x", y=26)
                nc.gpsimd.tensor_copy(x3[:, 1:25, 1:25], p3)
                nc.gpsimd.tensor_copy(x3[:, 0:1, 1:25], p3[:, 1:2, :])
                nc.gpsimd.tensor_copy(x3[:, 25:26, 1:25], p3[:, 22:23, :])
                nc.gpsimd.tensor_copy(x3[:, :, 0:1], x3[:, :, 2:3])
                nc.gpsimd.tensor_copy(x3[:, :, 25:26], x3[:, :, 23:24])

                xrep = convtmp.tile([96, 24 * 26], DT, tag="xrep")
                for dy in range(3):
                    nc.gpsimd.tensor_copy(
                        xrep[dy * 32 : (dy + 1) * 32, :],
                        xpad[:, dy * 26 : dy * 26 + 624],
                    )
                xr3 = xrep[:].rearrange("p (y x) -> p y x", y=OUT, x=26)

                for half in range(2):
                    pc = psum_pool.tile([32, 288], DT, tag="convps")
                    for dx in range(3):
                        nc.tensor.matmul(
                            pc[:],
                            wt_sb[:, (m * 3 + dx) * 32 : (m * 3 + dx + 1) * 32],
                            xr3[:, 12 * half : 12 * half + 12, dx : dx + 24],
                            start=(dx == 0),
                            stop=(dx == 2),
                        )
                    # LeakyReLU(0.2) == max(0.2*z, z); PSUM may feed only one
                    # non-scalar input, so stage a copy through SBUF first
                    zc = convtmp.tile([32, 288], DT, tag="zcopy")
                    nc.scalar.copy(zc[:], pc[:])
                    nc.vector.scalar_tensor_tensor(
                        out=v_sb[
                            m * 32 : (m + 1) * 32,
                            t * PIX + half * 288 : t * PIX + (half + 1) * 288,
                        ],
                        in0=zc[:],
                        scalar=0.2,
                        in1=pc[:],
                        op0=mybir.AluOpType.mult,
                        op1=mybir.AluOpType.max,
                    )

            if DEBUG_TAPS:
                nc.gpsimd.dma_start(out=pooled_out[0:128], in_=pooledA[:])
                nc.gpsimd.dma_start(out=pooled_out[128:192], in_=pooledB[:])
                nc.gpsimd.dma_start(out=v_out.ap(), in_=v_sb[:])

            # ---- Gram per t: transpose V chunks, then accumulate VT^T@VT ----
            for t in range(TPC):
                gp = psumg_pool.tile([96, 96], DT, tag="gram")
                for c in range(5):
                    sz = 128 if c < 4 else 64
                    vslice = v_sb[:, t * PIX + c * 128 : t * PIX + c * 128 + sz]
                    pt = psum_pool.tile([128, 96], DT, tag="vtps")
                    nc.tensor.transpose(pt[:sz, :], vslice, id_sb[:96, :96])
                    vt = vtpool.tile([128, 96], DT, tag="vt")
                    nc.scalar.copy(vt[:sz, :], pt[:sz, :])
                    nc.tensor.matmul(
                        gp[:], vt[:sz, :], vt[:sz, :],
                        start=(c == 0), stop=(c == 4),
                    )
                nc.scalar.copy(g_sb[:, t * 96 : (t + 1) * 96], gp[:])
                nc.gpsimd.dma_start(
                    out=g_out[t], in_=g_sb[:, t * 96 : (t + 1) * 96]
                )

    nc.finalize()
    return nc


def _get_nc():
    if "nc" not in _STATE:
        _STATE["nc"] = _build_nc()
    return _STATE["nc"]


def _prep_weights(W1, W2, W3):
    # wt[m, dx, dy*32+ic, oc] = W_m[oc, ic, dy, dx] / 64   (pool-mean folded in)
    wt = np.stack(
        [np.asarray(w, np.float64).transpose(3, 2, 1, 0).reshape(3, 96, 32)
         for w in (W1, W2, W3)]
    ) / 64.0
    return np.ascontiguousarray(wt, dtype=np.float32)


def _host_loss(G):
    G = np.asarray(G, np.float64)  # [16, 96, 96]
    T = G.shape[0]
    I96 = np.eye(M)
    Me = I96[None] + ALPHA_E * G
    ld_e = 2.0 * np.log(
        np.diagonal(np.linalg.cholesky(Me), axis1=-2, axis2=-1)
    ).sum()
    blocks = np.stack(
        [G[:, 32 * c : 32 * (c + 1), 32 * c : 32 * (c + 1)] for c in range(3)]
    )  # [3, T, 32, 32]
    Mc = np.eye(32)[None, None] + ALPHA_C * blocks
    ld_c = 2.0 * np.log(
        np.diagonal(np.linalg.cholesky(Mc), axis1=-2, axis2=-1)
    ).sum()
    loss_expd = ld_e / (2.0 * T)
    loss_comp = (32.0 / M) * ld_c / (2.0 * T)
    return np.float32(loss_expd - loss_comp)


def run_device(inputs, **kw):
    """Run the bass kernel; returns (G [16,96,96], BassKernelResults)."""
    from concourse.bass_utils import run_bass_kernel_spmd

    nc = _get_nc()
    wt = _prep_weights(inputs["W1"], inputs["W2"], inputs["W3"])
    ident = np.eye(128, dtype=np.float32)
    ms = np.asarray(inputs["ms_fea"], np.float32)
    pan = np.asarray(inputs["pan_fea"], np.float32)
    alf = np.asarray(inputs["all_fea"], np.float32)
    in_maps = []
    for i in range(NCORES):
        sl = slice(TPC * i, TPC * (i + 1))
        # x[t*3+m] = (ms,pan,alf)[m][t]
        xs = np.stack([ms[sl], pan[sl], alf[sl]], axis=1).reshape(
            TPC * 3, CCH, H, W
        )
        in_maps.append(
            {"x": np.ascontiguousarray(xs), "wt": wt, "ident": ident}
        )
    res = run_bass_kernel_spmd(nc, in_maps, core_ids=list(range(NCORES)), **kw)
    G = np.concatenate([np.asarray(r["g_out"]) for r in res.results], axis=0)
    return G, res


def kernel(**inputs):
    G, _ = run_device(inputs)
    return _host_loss(G)



# revision 5
# speedup vs baseline: 1.5867x; 1.5867x over previous
"""MCR loss kernel for Trainium2 (8 NeuronCores).

Strategy:
  - Shard batch T=16 -> 2 timesteps per core (data parallel, no collectives).
  - Host converts inputs to bf16: halves HBM traffic (the roofline term) and
    enables the DVE 2x packed mode for the pooling adds.
  - Per core, 6 plane-groups (2 timesteps x 3 maps).  Each group's 32x192x192
    plane stack is one contiguous 2.25MB DMA into [128=(c,quarter), 9216].
  - 8x8 avg-pool (as sum; 1/64 folded into conv weights) via a 6-op
    tensor_tensor pairwise tree on the vector engine (2x mode on bf16).
  - Reflect-pad + dy-replication built by small SBUF->SBUF DMAs on the
    scalar-engine HWDGE ring; 3x3 conv as 3 PE matmuls with K=(dy,ic)=96;
    LeakyReLU(0.2) natively on the scalar engine (Lrelu, PSUM->SBUF).
  - Gram G_t = V_t V_t^T via PE transpose + bf16 matmul (f32 PSUM accum).
  - Host: matrix determinant lemma
        logdet(I_576 + a V^T V) = logdet(I_96 + a V V^T)
    so only the [2,96,96] Grams leave the device; float64 Cholesky logdets
    finish the scalar loss.
"""

import numpy as np
import ml_dtypes

_STATE = {}

# -------- fixed problem geometry (hardcoded per harness contract) --------
B, CCH, H, W = 16, 32, 192, 192
NCORES = 8
TPC = B // NCORES          # timesteps per core = 2
NG = TPC * 3               # plane groups per core
OUT = 24                   # pooled spatial size
PIX = OUT * OUT            # 576
M = 96                     # feature rows (3 maps x 32 channels)
ALPHA_E = 6.0              # 576 / (96 * eps)
ALPHA_C = 18.0             # 576 / (32 * eps)

DEBUG_TAPS = False


def _build_nc():
    import concourse.bass as bass
    import concourse.tile as tile
    from concourse import bacc, mybir

    BF = mybir.dt.bfloat16
    F32 = mybir.dt.float32
    ACT = mybir.ActivationFunctionType
    OP = mybir.AluOpType

    nc = bacc.Bacc(
        "TRN2", target_bir_lowering=False, debug=False, num_devices=NCORES
    )

    x = nc.declare_dram_parameter("x", [NG, CCH, H, W], BF, isOutput=False)
    wt = nc.declare_dram_parameter("wt", [96, 288], BF, isOutput=False)
    ident = nc.declare_dram_parameter("ident", [96, 96], BF, isOutput=False)
    g_out = nc.declare_dram_parameter("g_out", [TPC, M, M], F32, isOutput=True)
    if DEBUG_TAPS:
        pooled_out = nc.declare_dram_parameter(
            "pooled_out", [NG * 32, PIX], F32, isOutput=True
        )
        v_out = nc.declare_dram_parameter("v_out", [96, TPC * PIX], F32, isOutput=True)

    with tile.TileContext(nc) as tc:
        with (
            tc.tile_pool(name="persist", bufs=1) as persist,
            tc.tile_pool(name="slab", bufs=2) as slab_pool,
            tc.tile_pool(name="red", bufs=2) as red_pool,
            tc.tile_pool(name="xrep", bufs=2) as xrep_pool,
            tc.tile_pool(name="vt", bufs=2) as vt_pool,
            tc.tile_pool(name="psc", bufs=2, space="PSUM") as psc_pool,
            tc.tile_pool(name="pst", bufs=2, space="PSUM") as pst_pool,
            tc.tile_pool(name="psg", bufs=2, space="PSUM") as psg_pool,
        ):
            wt_sb = persist.tile([96, 288], BF, tag="wt")
            nc.sync.dma_start(out=wt_sb[:], in_=wt.ap())
            id_sb = persist.tile([96, 96], BF, tag="id")
            nc.sync.dma_start(out=id_sb[:], in_=ident.ap())
            v_sb = persist.tile([96, TPC * PIX], BF, tag="v")
            g_sb = persist.tile([96, TPC * 96], F32, tag="g")

            for g in range(NG):
                t, m = divmod(g, 3)

                # ---- load: one contiguous 2.25MB slab, partition=(c, quarter)
                slab = slab_pool.tile([128, 9216], BF, tag="slab")
                nc.sync.dma_start(
                    out=slab[:],
                    in_=x.ap()[g].rearrange("c (q h) w -> (c q) (h w)", q=4),
                )

                # ---- pooling: pairwise tensor_tensor tree (bf16 2x mode) ----
                # per partition: 48 rows x 192 cols = (48h, 24x, 8w)
                sv = slab[:].rearrange("p (h x w) -> p h x w", h=48, x=24, w=8)
                t1 = red_pool.tile([128, 4608], BF, tag="t1")
                t1v = t1[:].rearrange("p (h x w) -> p h x w", h=48, x=24, w=4)
                nc.vector.tensor_tensor(
                    out=t1v, in0=sv[:, :, :, 0:4], in1=sv[:, :, :, 4:8], op=OP.add
                )
                t2 = red_pool.tile([128, 2304], BF, tag="t2")
                t2v = t2[:].rearrange("p (h x w) -> p h x w", h=48, x=24, w=2)
                nc.vector.tensor_tensor(
                    out=t2v, in0=t1v[:, :, :, 0:2], in1=t1v[:, :, :, 2:4], op=OP.add
                )
                # h-direction 8:1 before the final w-pair: (6y, 8r, 48xw)
                t2r = t2[:].rearrange("p (y r s) -> p y r s", y=6, r=8, s=48)
                t3 = red_pool.tile([128, 1152], BF, tag="t3")
                t3v = t3[:].rearrange("p (y r s) -> p y r s", y=6, r=4, s=48)
                nc.vector.tensor_tensor(
                    out=t3v, in0=t2r[:, :, 0:4, :], in1=t2r[:, :, 4:8, :], op=OP.add
                )
                t4 = red_pool.tile([128, 576], BF, tag="t4")
                t4v = t4[:].rearrange("p (y r s) -> p y r s", y=6, r=2, s=48)
                nc.vector.tensor_tensor(
                    out=t4v, in0=t3v[:, :, 0:2, :], in1=t3v[:, :, 2:4, :], op=OP.add
                )
                t5 = red_pool.tile([128, 288], BF, tag="t5")
                t5v = t5[:].rearrange("p (y r s) -> p y r s", y=6, r=1, s=48)
                nc.vector.tensor_tensor(
                    out=t5v, in0=t4v[:, :, 0:1, :], in1=t4v[:, :, 1:2, :], op=OP.add
                )
                # final w-pair: (6y, 24x, 2w) -> pooled [128=(c,q), (6y,24x)]
                t5w = t5[:].rearrange("p (y x w) -> p y x w", y=6, x=24, w=2)
                pooled = red_pool.tile([128, 144], BF, tag="pooled")
                pv0 = pooled[:].rearrange("p (y x w) -> p y x w", y=6, x=24, w=1)
                nc.vector.tensor_tensor(
                    out=pv0, in0=t5w[:, :, :, 0:1], in1=t5w[:, :, :, 1:2], op=OP.add
                )

                # ---- gather quarters: [128=(c,q),144] -> [32=c, 576] --------
                pooledT = red_pool.tile([32, PIX], BF, tag="pooledT")
                nc.scalar.dma_start(out=pooledT[:], in_=pooled[:])
                pvw = pooledT[:].rearrange("c (y x) -> c y x", y=OUT)

                if DEBUG_TAPS:
                    nc.gpsimd.dma_start(
                        out=pooled_out.ap()[g * 32 : (g + 1) * 32], in_=pooledT[:]
                    )

                # ---- build xrep [96=(dy,c), 24y, 26x]: dy-shifted, padded ---
                xrep = xrep_pool.tile([96, 24 * 26], BF, tag="xrep")
                xr3 = xrep[:].rearrange("p (y x) -> p y x", y=OUT, x=26)
                # dy=1: rows 0..23 straight
                nc.scalar.dma_start(out=xr3[32:64, :, 1:25], in_=pvw[:, :, :])
                # dy=0: rows -1..22 with reflect(-1)=row1
                nc.scalar.dma_start(out=xr3[0:32, 1:24, 1:25], in_=pvw[:, 0:23, :])
                nc.scalar.dma_start(out=xr3[0:32, 0:1, 1:25], in_=pvw[:, 1:2, :])
                # dy=2: rows 1..24 with reflect(24)=row22
                nc.scalar.dma_start(out=xr3[64:96, 0:23, 1:25], in_=pvw[:, 1:24, :])
                nc.scalar.dma_start(out=xr3[64:96, 23:24, 1:25], in_=pvw[:, 22:23, :])
                # x reflect pads (lane-local)
                nc.scalar.copy(out=xr3[:, :, 0:1], in_=xr3[:, :, 2:3])
                nc.scalar.copy(out=xr3[:, :, 25:26], in_=xr3[:, :, 23:24])

                # ---- conv: 2 halves x 3 dx matmuls, K=(dy,ic)=96 ------------
                for half in range(2):
                    pc = psc_pool.tile([32, 288], F32, tag="convps")
                    for dx in range(3):
                        nc.tensor.matmul(
                            pc[:],
                            wt_sb[:, (m * 3 + dx) * 32 : (m * 3 + dx + 1) * 32],
                            xr3[:, 12 * half : 12 * half + 12, dx : dx + 24],
                            start=(dx == 0),
                            stop=(dx == 2),
                        )
                    # LeakyReLU(0.2) on the scalar engine, PSUM -> SBUF bf16
                    nc.scalar.activation(
                        out=v_sb[
                            m * 32 : (m + 1) * 32,
                            t * PIX + half * 288 : t * PIX + (half + 1) * 288,
                        ],
                        in_=pc[:],
                        func=ACT.Prelu,
                        alpha=0.2,
                    )

                # ---- Gram per t once its 3 maps are done --------------------
                if m == 2:
                    if DEBUG_TAPS:
                        nc.gpsimd.dma_start(
                            out=v_out.ap()[:, t * PIX : (t + 1) * PIX],
                            in_=v_sb[:, t * PIX : (t + 1) * PIX],
                        )
                    gp = psg_pool.tile([96, 96], F32, tag="gram")
                    for c5 in range(5):
                        sz = 128 if c5 < 4 else 64
                        vsl = v_sb[:, t * PIX + c5 * 128 : t * PIX + c5 * 128 + sz]
                        pt = pst_pool.tile([128, 96], BF, tag="vtps")
                        nc.tensor.transpose(pt[:sz, :], vsl, id_sb[:])
                        vtt = vt_pool.tile([128, 96], BF, tag="vt")
                        nc.scalar.copy(out=vtt[:sz, :], in_=pt[:sz, :])
                        nc.tensor.matmul(
                            gp[:], vtt[:sz, :], vtt[:sz, :],
                            start=(c5 == 0), stop=(c5 == 4),
                        )
                    nc.scalar.copy(out=g_sb[:, t * 96 : (t + 1) * 96], in_=gp[:])
                    nc.sync.dma_start(
                        out=g_out.ap()[t], in_=g_sb[:, t * 96 : (t + 1) * 96]
                    )

    nc.finalize()
    return nc


def _get_nc():
    if "nc" not in _STATE:
        _STATE["nc"] = _build_nc()
    return _STATE["nc"]


def _prep_weights(W1, W2, W3):
    # wt[(dy,ic), (m,dx,oc)] = W_m[oc, ic, dy, dx] / 64   (pool-mean folded in)
    w = np.stack([np.asarray(Wi, np.float64) for Wi in (W1, W2, W3)])
    wt = w.transpose(3, 2, 0, 4, 1).reshape(96, 288) / 64.0
    return wt.astype(ml_dtypes.bfloat16)


def _host_loss(G):
    G = np.asarray(G, np.float64)  # [16, 96, 96]
    T = G.shape[0]
    I96 = np.eye(M)
    Me = I96[None] + ALPHA_E * G
    ld_e = 2.0 * np.log(
        np.diagonal(np.linalg.cholesky(Me), axis1=-2, axis2=-1)
    ).sum()
    blocks = np.stack(
        [G[:, 32 * c : 32 * (c + 1), 32 * c : 32 * (c + 1)] for c in range(3)]
    )  # [3, T, 32, 32]
    Mc = np.eye(32)[None, None] + ALPHA_C * blocks
    ld_c = 2.0 * np.log(
        np.diagonal(np.linalg.cholesky(Mc), axis1=-2, axis2=-1)
    ).sum()
    loss_expd = ld_e / (2.0 * T)
    loss_comp = (32.0 / M) * ld_c / (2.0 * T)
    return np.float32(loss_expd - loss_comp)


def run_device(inputs, **kw):
    """Run the bass kernel; returns (G [16,96,96], BassKernelResults)."""
    from concourse.bass_utils import run_bass_kernel_spmd

    nc = _get_nc()
    wt = _prep_weights(inputs["W1"], inputs["W2"], inputs["W3"])
    ident = np.eye(96, dtype=ml_dtypes.bfloat16)
    ms = np.asarray(inputs["ms_fea"], np.float32)
    pan = np.asarray(inputs["pan_fea"], np.float32)
    alf = np.asarray(inputs["all_fea"], np.float32)
    in_maps = []
    for i in range(NCORES):
        sl = slice(TPC * i, TPC * (i + 1))
        # x[t*3+m] = (ms,pan,alf)[m][t]
        xs = np.stack([ms[sl], pan[sl], alf[sl]], axis=1).reshape(
            NG, CCH, H, W
        )
        in_maps.append(
            {"x": xs.astype(ml_dtypes.bfloat16), "wt": wt, "ident": ident}
        )
    res = run_bass_kernel_spmd(nc, in_maps, core_ids=list(range(NCORES)), **kw)
    G = np.concatenate([np.asarray(r["g_out"]) for r in res.results], axis=0)
    return G, res


def kernel(**inputs):
    G, _ = run_device(inputs)
    return _host_loss(G)


# revision 6
# speedup vs baseline: 1.9026x; 1.1991x over previous
"""MCR loss kernel for Trainium2 (8 NeuronCores).

Strategy:
  - Shard batch T=16 -> 2 timesteps per core (data parallel, no collectives).
  - Host converts inputs to bf16: halves HBM traffic (the roofline term) and
    enables the DVE 2x packed mode for the pooling adds.
  - Per core, 6 plane-groups (2 timesteps x 3 maps).  Each group's 32x192x192
    plane stack is one contiguous 2.25MB DMA into [128=(c,quarter), 9216].
  - 8x8 avg-pool (as sum; 1/64 folded into conv weights) via a 6-op
    tensor_tensor pairwise tree on the vector engine (2x mode on bf16).
  - Reflect-pad + dy-replication built by small SBUF->SBUF DMAs on the
    scalar-engine HWDGE ring; 3x3 conv as 3 PE matmuls with K=(dy,ic)=96;
    LeakyReLU(0.2) natively on the scalar engine (Lrelu, PSUM->SBUF).
  - Gram G_t = V_t V_t^T via PE transpose + bf16 matmul (f32 PSUM accum).
  - Host: matrix determinant lemma
        logdet(I_576 + a V^T V) = logdet(I_96 + a V V^T)
    so only the [2,96,96] Grams leave the device; float64 Cholesky logdets
    finish the scalar loss.
"""

import numpy as np
import ml_dtypes

_STATE = {}

# -------- fixed problem geometry (hardcoded per harness contract) --------
B, CCH, H, W = 16, 32, 192, 192
NCORES = 8
TPC = B // NCORES          # timesteps per core = 2
NG = TPC * 3               # plane groups per core
OUT = 24                   # pooled spatial size
PIX = OUT * OUT            # 576
M = 96                     # feature rows (3 maps x 32 channels)
ALPHA_E = 6.0              # 576 / (96 * eps)
ALPHA_C = 18.0             # 576 / (32 * eps)

DEBUG_TAPS = False


def _build_nc():
    import concourse.bass as bass
    import concourse.tile as tile
    from concourse import bacc, mybir

    BF = mybir.dt.bfloat16
    F32 = mybir.dt.float32
    ACT = mybir.ActivationFunctionType
    OP = mybir.AluOpType

    nc = bacc.Bacc(
        "TRN2", target_bir_lowering=False, debug=False, num_devices=NCORES
    )

    x = nc.declare_dram_parameter("x", [NG, CCH, H, W], BF, isOutput=False)
    wt = nc.declare_dram_parameter("wt", [96, 288], BF, isOutput=False)
    ident = nc.declare_dram_parameter("ident", [96, 96], BF, isOutput=False)
    g_out = nc.declare_dram_parameter("g_out", [TPC, M, M], F32, isOutput=True)
    if DEBUG_TAPS:
        pooled_out = nc.declare_dram_parameter(
            "pooled_out", [NG * 32, PIX], F32, isOutput=True
        )
        v_out = nc.declare_dram_parameter("v_out", [96, TPC * PIX], F32, isOutput=True)

    with tile.TileContext(nc) as tc:
        with (
            tc.tile_pool(name="persist", bufs=1) as persist,
            tc.tile_pool(name="slab", bufs=2) as slab_pool,
            tc.tile_pool(name="red", bufs=2) as red_pool,
            tc.tile_pool(name="xrep", bufs=2) as xrep_pool,
            tc.tile_pool(name="vt", bufs=2) as vt_pool,
            tc.tile_pool(name="psc", bufs=2, space="PSUM") as psc_pool,
            tc.tile_pool(name="pst", bufs=2, space="PSUM") as pst_pool,
            tc.tile_pool(name="psg", bufs=2, space="PSUM") as psg_pool,
        ):
            wt_sb = persist.tile([96, 288], BF, tag="wt")
            nc.sync.dma_start(out=wt_sb[:], in_=wt.ap())
            id_sb = persist.tile([96, 96], BF, tag="id")
            nc.sync.dma_start(out=id_sb[:], in_=ident.ap())
            v_sb = persist.tile([96, TPC * PIX], BF, tag="v")
            g_sb = persist.tile([96, TPC * 96], F32, tag="g")

            for g in range(NG):
                t, m = divmod(g, 3)

                # ---- load: one contiguous 2.25MB slab, partition=(c, quarter)
                slab = slab_pool.tile([128, 9216], BF, tag="slab")
                nc.sync.dma_start(
                    out=slab[:],
                    in_=x.ap()[g].rearrange("c (q h) w -> (c q) (h w)", q=4),
                )

                # ---- pooling: pairwise tensor_tensor tree (bf16 2x mode) ----
                # per partition: 48 rows x 192 cols = (48h, 24x, 8w)
                sv = slab[:].rearrange("p (h x w) -> p h x w", h=48, x=24, w=8)
                t1 = red_pool.tile([128, 4608], BF, tag="t1")
                t1v = t1[:].rearrange("p (h x w) -> p h x w", h=48, x=24, w=4)
                nc.vector.tensor_tensor(
                    out=t1v, in0=sv[:, :, :, 0:4], in1=sv[:, :, :, 4:8], op=OP.add
                )
                t2 = red_pool.tile([128, 2304], BF, tag="t2")
                t2v = t2[:].rearrange("p (h x w) -> p h x w", h=48, x=24, w=2)
                nc.vector.tensor_tensor(
                    out=t2v, in0=t1v[:, :, :, 0:2], in1=t1v[:, :, :, 2:4], op=OP.add
                )
                # h-direction 8:1 before the final w-pair: (6y, 8r, 48xw)
                t2r = t2[:].rearrange("p (y r s) -> p y r s", y=6, r=8, s=48)
                t3 = red_pool.tile([128, 1152], BF, tag="t3")
                t3v = t3[:].rearrange("p (y r s) -> p y r s", y=6, r=4, s=48)
                nc.vector.tensor_tensor(
                    out=t3v, in0=t2r[:, :, 0:4, :], in1=t2r[:, :, 4:8, :], op=OP.add
                )
                t4 = red_pool.tile([128, 576], BF, tag="t4")
                t4v = t4[:].rearrange("p (y r s) -> p y r s", y=6, r=2, s=48)
                nc.vector.tensor_tensor(
                    out=t4v, in0=t3v[:, :, 0:2, :], in1=t3v[:, :, 2:4, :], op=OP.add
                )
                t5 = red_pool.tile([128, 288], BF, tag="t5")
                t5v = t5[:].rearrange("p (y r s) -> p y r s", y=6, r=1, s=48)
                nc.vector.tensor_tensor(
                    out=t5v, in0=t4v[:, :, 0:1, :], in1=t4v[:, :, 1:2, :], op=OP.add
                )
                # final w-pair written straight into the x-padded 26-wide row
                # layout: pooled [128=(c,q), (6y, 26x)], cols 1..24 are data
                t5w = t5[:].rearrange("p (y x w) -> p y x w", y=6, x=24, w=2)
                pooled = red_pool.tile([128, 6 * 26], BF, tag="pooled")
                pv26 = pooled[:].rearrange("p (y x) -> p y x", y=6, x=26)
                pv0 = pv26[:, :, 1:25].rearrange("p y (x w) -> p y x w", w=1)
                nc.vector.tensor_tensor(
                    out=pv0, in0=t5w[:, :, :, 0:1], in1=t5w[:, :, :, 1:2], op=OP.add
                )
                # x reflect pads, lane-local (cols 0,25 <- cols 2,23)
                nc.gpsimd.tensor_copy(pv26[:, :, 0:1], pv26[:, :, 2:3])
                nc.gpsimd.tensor_copy(pv26[:, :, 25:26], pv26[:, :, 23:24])

                # ---- build xrep [96=(dy,c), 24y, 26x]: all-contiguous DMAs --
                xrep = xrep_pool.tile([96, 24 * 26], BF, tag="xrep")
                # gather quarters straight into the dy=1 block
                nc.scalar.dma_start(out=xrep[32:64, :], in_=pooled[:])
                # dy=0: row slot y holds source row y-1; reflect(-1)=row1
                nc.scalar.dma_start(
                    out=xrep[0:32, 26:624], in_=xrep[32:64, 0:598]
                )
                nc.scalar.dma_start(out=xrep[0:32, 0:26], in_=xrep[32:64, 26:52])
                # dy=2: row slot y holds source row y+1; reflect(24)=row22
                nc.scalar.dma_start(
                    out=xrep[64:96, 0:598], in_=xrep[32:64, 26:624]
                )
                nc.scalar.dma_start(
                    out=xrep[64:96, 598:624], in_=xrep[32:64, 572:598]
                )
                xr3 = xrep[:].rearrange("p (y x) -> p y x", y=OUT, x=26)

                if DEBUG_TAPS:
                    nc.gpsimd.dma_start(
                        out=pooled_out.ap()[g * 32 : (g + 1) * 32],
                        in_=xr3[32:64, :, 1:25],
                    )

                # ---- conv: 2 halves x 3 dx matmuls, K=(dy,ic)=96 ------------
                for half in range(2):
                    pc = psc_pool.tile([32, 288], F32, tag="convps")
                    for dx in range(3):
                        nc.tensor.matmul(
                            pc[:],
                            wt_sb[:, (m * 3 + dx) * 32 : (m * 3 + dx + 1) * 32],
                            xr3[:, 12 * half : 12 * half + 12, dx : dx + 24],
                            start=(dx == 0),
                            stop=(dx == 2),
                        )
                    # LeakyReLU(0.2) on the scalar engine, PSUM -> SBUF bf16
                    nc.scalar.activation(
                        out=v_sb[
                            m * 32 : (m + 1) * 32,
                            t * PIX + half * 288 : t * PIX + (half + 1) * 288,
                        ],
                        in_=pc[:],
                        func=ACT.Prelu,
                        alpha=0.2,
                    )

                # ---- Gram per t once its 3 maps are done --------------------
                if m == 2:
                    if DEBUG_TAPS:
                        nc.gpsimd.dma_start(
                            out=v_out.ap()[:, t * PIX : (t + 1) * PIX],
                            in_=v_sb[:, t * PIX : (t + 1) * PIX],
                        )
                    gp = psg_pool.tile([96, 96], F32, tag="gram")
                    for c5 in range(5):
                        sz = 128 if c5 < 4 else 64
                        vsl = v_sb[:, t * PIX + c5 * 128 : t * PIX + c5 * 128 + sz]
                        pt = pst_pool.tile([128, 96], BF, tag="vtps")
                        nc.tensor.transpose(pt[:sz, :], vsl, id_sb[:])
                        vtt = vt_pool.tile([128, 96], BF, tag="vt")
                        nc.scalar.copy(out=vtt[:sz, :], in_=pt[:sz, :])
                        nc.tensor.matmul(
                            gp[:], vtt[:sz, :], vtt[:sz, :],
                            start=(c5 == 0), stop=(c5 == 4),
                        )
                    nc.scalar.copy(out=g_sb[:, t * 96 : (t + 1) * 96], in_=gp[:])
                    nc.sync.dma_start(
                        out=g_out.ap()[t], in_=g_sb[:, t * 96 : (t + 1) * 96]
                    )

    nc.finalize()
    return nc


def _get_nc():
    if "nc" not in _STATE:
        _STATE["nc"] = _build_nc()
    return _STATE["nc"]


def _prep_weights(W1, W2, W3):
    # wt[(dy,ic), (m,dx,oc)] = W_m[oc, ic, dy, dx] / 64   (pool-mean folded in)
    w = np.stack([np.asarray(Wi, np.float64) for Wi in (W1, W2, W3)])
    wt = w.transpose(3, 2, 0, 4, 1).reshape(96, 288) / 64.0
    return wt.astype(ml_dtypes.bfloat16)


def _host_loss(G):
    G = np.asarray(G, np.float64)  # [16, 96, 96]
    T = G.shape[0]
    I96 = np.eye(M)
    Me = I96[None] + ALPHA_E * G
    ld_e = 2.0 * np.log(
        np.diagonal(np.linalg.cholesky(Me), axis1=-2, axis2=-1)
    ).sum()
    blocks = np.stack(
        [G[:, 32 * c : 32 * (c + 1), 32 * c : 32 * (c + 1)] for c in range(3)]
    )  # [3, T, 32, 32]
    Mc = np.eye(32)[None, None] + ALPHA_C * blocks
    ld_c = 2.0 * np.log(
        np.diagonal(np.linalg.cholesky(Mc), axis1=-2, axis2=-1)
    ).sum()
    loss_expd = ld_e / (2.0 * T)
    loss_comp = (32.0 / M) * ld_c / (2.0 * T)
    return np.float32(loss_expd - loss_comp)


def run_device(inputs, **kw):
    """Run the bass kernel; returns (G [16,96,96], BassKernelResults)."""
    from concourse.bass_utils import run_bass_kernel_spmd

    nc = _get_nc()
    wt = _prep_weights(inputs["W1"], inputs["W2"], inputs["W3"])
    ident = np.eye(96, dtype=ml_dtypes.bfloat16)
    ms = np.asarray(inputs["ms_fea"], np.float32)
    pan = np.asarray(inputs["pan_fea"], np.float32)
    alf = np.asarray(inputs["all_fea"], np.float32)
    in_maps = []
    for i in range(NCORES):
        sl = slice(TPC * i, TPC * (i + 1))
        # x[t*3+m] = (ms,pan,alf)[m][t]
        xs = np.stack([ms[sl], pan[sl], alf[sl]], axis=1).reshape(
            NG, CCH, H, W
        )
        in_maps.append(
            {"x": xs.astype(ml_dtypes.bfloat16), "wt": wt, "ident": ident}
        )
    res = run_bass_kernel_spmd(nc, in_maps, core_ids=list(range(NCORES)), **kw)
    G = np.concatenate([np.asarray(r["g_out"]) for r in res.results], axis=0)
    return G, res


def kernel(**inputs):
    G, _ = run_device(inputs)
    return _host_loss(G)


# revision 9
# speedup vs baseline: 1.9822x; 1.0419x over previous
"""MCR loss kernel for Trainium2 (8 NeuronCores).

Strategy:
  - Shard batch T=16 -> 2 timesteps per core (data parallel, no collectives).
  - Host converts inputs to bf16: halves HBM traffic (the roofline term) and
    enables the DVE 2x packed mode for the pooling adds.
  - Per core, 6 plane-groups (2 timesteps x 3 maps).  Each group's 32x192x192
    plane stack is one contiguous 2.25MB DMA into [128=(c,quarter), 9216].
  - 8x8 avg-pool (as sum; 1/64 folded into conv weights) via a 6-op
    tensor_tensor pairwise tree on the vector engine (2x mode on bf16).
  - Reflect-pad + dy-replication built by small SBUF->SBUF DMAs on the
    scalar-engine HWDGE ring; 3x3 conv as 3 PE matmuls with K=(dy,ic)=96;
    LeakyReLU(0.2) natively on the scalar engine (Lrelu, PSUM->SBUF).
  - Gram G_t = V_t V_t^T via PE transpose + bf16 matmul (f32 PSUM accum).
  - Host: matrix determinant lemma
        logdet(I_576 + a V^T V) = logdet(I_96 + a V V^T)
    so only the [2,96,96] Grams leave the device; float64 Cholesky logdets
    finish the scalar loss.
"""

import numpy as np
import ml_dtypes

_STATE = {}

# -------- fixed problem geometry (hardcoded per harness contract) --------
B, CCH, H, W = 16, 32, 192, 192
NCORES = 8
TPC = B // NCORES          # timesteps per core = 2
NG = TPC * 3               # plane groups per core
OUT = 24                   # pooled spatial size
PIX = OUT * OUT            # 576
M = 96                     # feature rows (3 maps x 32 channels)
ALPHA_E = 6.0              # 576 / (96 * eps)
ALPHA_C = 18.0             # 576 / (32 * eps)

DEBUG_TAPS = False


def _build_nc():
    import concourse.bass as bass
    import concourse.tile as tile
    from concourse import bacc, mybir

    BF = mybir.dt.bfloat16
    F32 = mybir.dt.float32
    ACT = mybir.ActivationFunctionType
    OP = mybir.AluOpType

    nc = bacc.Bacc(
        "TRN2", target_bir_lowering=False, debug=False, num_devices=NCORES
    )

    x = nc.declare_dram_parameter("x", [NG, CCH, H, W], BF, isOutput=False)
    wt = nc.declare_dram_parameter("wt", [96, 288], BF, isOutput=False)
    ident = nc.declare_dram_parameter("ident", [96, 96], BF, isOutput=False)
    g_out = nc.declare_dram_parameter("g_out", [TPC, M, M], F32, isOutput=True)
    if DEBUG_TAPS:
        pooled_out = nc.declare_dram_parameter(
            "pooled_out", [NG * 32, PIX], F32, isOutput=True
        )
        v_out = nc.declare_dram_parameter("v_out", [96, TPC * PIX], F32, isOutput=True)

    with tile.TileContext(nc) as tc:
        with (
            tc.tile_pool(name="persist", bufs=1) as persist,
            tc.tile_pool(name="slab", bufs=3) as slab_pool,
            tc.tile_pool(name="red", bufs=2) as red_pool,
            tc.tile_pool(name="xrep", bufs=2) as xrep_pool,
            tc.tile_pool(name="vt", bufs=2) as vt_pool,
            tc.tile_pool(name="psc", bufs=2, space="PSUM") as psc_pool,
            tc.tile_pool(name="pst", bufs=2, space="PSUM") as pst_pool,
            tc.tile_pool(name="psg", bufs=2, space="PSUM") as psg_pool,
        ):
            # wt/id go on the scalar HWDGE ring so slab 0 is first on sync
            wt_sb = persist.tile([96, 288], BF, tag="wt")
            nc.scalar.dma_start(out=wt_sb[:], in_=wt.ap())
            id_sb = persist.tile([96, 96], BF, tag="id")
            nc.scalar.dma_start(out=id_sb[:], in_=ident.ap())
            v_sb = persist.tile([96, TPC * PIX], BF, tag="v")
            g_sb = persist.tile([96, TPC * 96], F32, tag="g")

            for g in range(NG):
                t, m = divmod(g, 3)

                # ---- load: one contiguous 2.25MB slab, partition=(c, quarter)
                slab = slab_pool.tile([128, 9216], BF, tag="slab")
                nc.sync.dma_start(
                    out=slab[:],
                    in_=x.ap()[g].rearrange("c (q h) w -> (c q) (h w)", q=4),
                )

                # ---- pooling: pairwise tensor_tensor tree (bf16 2x mode) ----
                # per partition: 48 rows x 192 cols = (48h, 24x, 8w)
                sv = slab[:].rearrange("p (h x w) -> p h x w", h=48, x=24, w=8)
                t1 = red_pool.tile([128, 4608], BF, tag="t1")
                t1v = t1[:].rearrange("p (h x w) -> p h x w", h=48, x=24, w=4)
                nc.vector.tensor_tensor(
                    out=t1v, in0=sv[:, :, :, 0:4], in1=sv[:, :, :, 4:8], op=OP.add
                )
                t2 = red_pool.tile([128, 2304], BF, tag="t2")
                t2v = t2[:].rearrange("p (h x w) -> p h x w", h=48, x=24, w=2)
                nc.vector.tensor_tensor(
                    out=t2v, in0=t1v[:, :, :, 0:2], in1=t1v[:, :, :, 2:4], op=OP.add
                )
                # h-direction 8:1 before the final w-pair: (6y, 8r, 48xw)
                t2r = t2[:].rearrange("p (y r s) -> p y r s", y=6, r=8, s=48)
                t3 = red_pool.tile([128, 1152], BF, tag="t3")
                t3v = t3[:].rearrange("p (y r s) -> p y r s", y=6, r=4, s=48)
                nc.vector.tensor_tensor(
                    out=t3v, in0=t2r[:, :, 0:4, :], in1=t2r[:, :, 4:8, :], op=OP.add
                )
                t4 = red_pool.tile([128, 576], BF, tag="t4")
                t4v = t4[:].rearrange("p (y r s) -> p y r s", y=6, r=2, s=48)
                nc.vector.tensor_tensor(
                    out=t4v, in0=t3v[:, :, 0:2, :], in1=t3v[:, :, 2:4, :], op=OP.add
                )
                t5 = red_pool.tile([128, 288], BF, tag="t5")
                t5v = t5[:].rearrange("p (y r s) -> p y r s", y=6, r=1, s=48)
                nc.vector.tensor_tensor(
                    out=t5v, in0=t4v[:, :, 0:1, :], in1=t4v[:, :, 1:2, :], op=OP.add
                )
                # final w-pair written straight into the x-padded 26-wide row
                # layout: pooled [128=(c,q), (6y, 26x)], cols 1..24 are data
                t5w = t5[:].rearrange("p (y x w) -> p y x w", y=6, x=24, w=2)
                pooled = red_pool.tile([128, 6 * 26], BF, tag="pooled")
                pv26 = pooled[:].rearrange("p (y x) -> p y x", y=6, x=26)
                pv0 = pv26[:, :, 1:25].rearrange("p y (x w) -> p y x w", w=1)
                nc.vector.tensor_tensor(
                    out=pv0, in0=t5w[:, :, :, 0:1], in1=t5w[:, :, :, 1:2], op=OP.add
                )
                # x reflect pads, lane-local (cols 0,25 <- cols 2,23).
                # On the scalar engine: gpsimd crawls here (SBUF-port
                # contention with concurrent 2-port DVE ops).
                nc.scalar.copy(out=pv26[:, :, 0:1], in_=pv26[:, :, 2:3])
                nc.scalar.copy(out=pv26[:, :, 25:26], in_=pv26[:, :, 23:24])

                # ---- build xrep [96=(dy,c), 24y, 26x]: all-contiguous DMAs --
                xrep = xrep_pool.tile([96, 24 * 26], BF, tag="xrep")
                # gather quarters straight into the dy=1 block
                nc.scalar.dma_start(out=xrep[32:64, :], in_=pooled[:])
                # dy=0: row slot y holds source row y-1; reflect(-1)=row1
                nc.scalar.dma_start(
                    out=xrep[0:32, 26:624], in_=xrep[32:64, 0:598]
                )
                nc.scalar.dma_start(out=xrep[0:32, 0:26], in_=xrep[32:64, 26:52])
                # dy=2: row slot y holds source row y+1; reflect(24)=row22
                nc.scalar.dma_start(
                    out=xrep[64:96, 0:598], in_=xrep[32:64, 26:624]
                )
                nc.scalar.dma_start(
                    out=xrep[64:96, 598:624], in_=xrep[32:64, 572:598]
                )
                xr3 = xrep[:].rearrange("p (y x) -> p y x", y=OUT, x=26)

                if DEBUG_TAPS:
                    nc.gpsimd.dma_start(
                        out=pooled_out.ap()[g * 32 : (g + 1) * 32],
                        in_=xr3[32:64, :, 1:25],
                    )

                # ---- conv: 2 halves x 3 dx matmuls, K=(dy,ic)=96 ------------
                for half in range(2):
                    pc = psc_pool.tile([32, 288], F32, tag="convps")
                    for dx in range(3):
                        nc.tensor.matmul(
                            pc[:],
                            wt_sb[:, (m * 3 + dx) * 32 : (m * 3 + dx + 1) * 32],
                            xr3[:, 12 * half : 12 * half + 12, dx : dx + 24],
                            start=(dx == 0),
                            stop=(dx == 2),
                        )
                    # LeakyReLU(0.2) on the scalar engine, PSUM -> SBUF bf16
                    nc.scalar.activation(
                        out=v_sb[
                            m * 32 : (m + 1) * 32,
                            t * PIX + half * 288 : t * PIX + (half + 1) * 288,
                        ],
                        in_=pc[:],
                        func=ACT.Prelu,
                        alpha=0.2,
                    )

                # ---- Gram per t once its 3 maps are done --------------------
                if m == 2:
                    if DEBUG_TAPS:
                        nc.gpsimd.dma_start(
                            out=v_out.ap()[:, t * PIX : (t + 1) * PIX],
                            in_=v_sb[:, t * PIX : (t + 1) * PIX],
                        )
                    gp = psg_pool.tile([96, 96], F32, tag="gram")
                    for c5 in range(5):
                        sz = 128 if c5 < 4 else 64
                        vsl = v_sb[:, t * PIX + c5 * 128 : t * PIX + c5 * 128 + sz]
                        pt = pst_pool.tile([128, 96], BF, tag="vtps")
                        nc.tensor.transpose(pt[:sz, :], vsl, id_sb[:])
                        vtt = vt_pool.tile([128, 96], BF, tag="vt")
                        nc.scalar.copy(out=vtt[:sz, :], in_=pt[:sz, :])
                        nc.tensor.matmul(
                            gp[:], vtt[:sz, :], vtt[:sz, :],
                            start=(c5 == 0), stop=(c5 == 4),
                        )
                    nc.scalar.copy(out=g_sb[:, t * 96 : (t + 1) * 96], in_=gp[:])
                    nc.sync.dma_start(
                        out=g_out.ap()[t], in_=g_sb[:, t * 96 : (t + 1) * 96]
                    )

    nc.finalize()
    return nc


def _get_nc():
    if "nc" not in _STATE:
        _STATE["nc"] = _build_nc()
    return _STATE["nc"]


def _prep_weights(W1, W2, W3):
    # wt[(dy,ic), (m,dx,oc)] = W_m[oc, ic, dy, dx] / 64   (pool-mean folded in)
    w = np.stack([np.asarray(Wi, np.float64) for Wi in (W1, W2, W3)])
    wt = w.transpose(3, 2, 0, 4, 1).reshape(96, 288) / 64.0
    return wt.astype(ml_dtypes.bfloat16)


def _host_loss(G):
    G = np.asarray(G, np.float64)  # [16, 96, 96]
    T = G.shape[0]
    I96 = np.eye(M)
    Me = I96[None] + ALPHA_E * G
    ld_e = 2.0 * np.log(
        np.diagonal(np.linalg.cholesky(Me), axis1=-2, axis2=-1)
    ).sum()
    blocks = np.stack(
        [G[:, 32 * c : 32 * (c + 1), 32 * c : 32 * (c + 1)] for c in range(3)]
    )  # [3, T, 32, 32]
    Mc = np.eye(32)[None, None] + ALPHA_C * blocks
    ld_c = 2.0 * np.log(
        np.diagonal(np.linalg.cholesky(Mc), axis1=-2, axis2=-1)
    ).sum()
    loss_expd = ld_e / (2.0 * T)
    loss_comp = (32.0 / M) * ld_c / (2.0 * T)
    return np.float32(loss_expd - loss_comp)


def run_device(inputs, **kw):
    """Run the bass kernel; returns (G [16,96,96], BassKernelResults)."""
    from concourse.bass_utils import run_bass_kernel_spmd

    nc = _get_nc()
    wt = _prep_weights(inputs["W1"], inputs["W2"], inputs["W3"])
    ident = np.eye(96, dtype=ml_dtypes.bfloat16)
    ms = np.asarray(inputs["ms_fea"], np.float32)
    pan = np.asarray(inputs["pan_fea"], np.float32)
    alf = np.asarray(inputs["all_fea"], np.float32)
    in_maps = []
    for i in range(NCORES):
        sl = slice(TPC * i, TPC * (i + 1))
        # x[t*3+m] = (ms,pan,alf)[m][t]
        xs = np.stack([ms[sl], pan[sl], alf[sl]], axis=1).reshape(
            NG, CCH, H, W
        )
        in_maps.append(
            {"x": xs.astype(ml_dtypes.bfloat16), "wt": wt, "ident": ident}
        )
    res = run_bass_kernel_spmd(nc, in_maps, core_ids=list(range(NCORES)), **kw)
    G = np.concatenate([np.asarray(r["g_out"]) for r in res.results], axis=0)
    return G, res


def kernel(**inputs):
    G, _ = run_device(inputs)
    return _host_loss(G)


# revision 11
# speedup vs baseline: 2.0059x; 1.0119x over previous
"""MCR loss kernel for Trainium2 (8 NeuronCores).

Strategy:
  - Shard batch T=16 -> 2 timesteps per core (data parallel, no collectives).
  - Host converts inputs to bf16: halves HBM traffic (the roofline term) and
    enables the DVE 2x packed mode for the pooling adds.
  - Per core, 6 plane-groups (2 timesteps x 3 maps).  Each group's 32x192x192
    plane stack is one contiguous 2.25MB DMA into [128=(c,quarter), 9216].
  - 8x8 avg-pool (as sum; 1/64 folded into conv weights) via a 6-op
    tensor_tensor pairwise tree on the vector engine (2x mode on bf16).
  - Reflect-pad + dy-replication built by small SBUF->SBUF DMAs on the
    scalar-engine HWDGE ring; 3x3 conv as 3 PE matmuls with K=(dy,ic)=96;
    LeakyReLU(0.2) natively on the scalar engine (Lrelu, PSUM->SBUF).
  - Gram G_t = V_t V_t^T via PE transpose + bf16 matmul (f32 PSUM accum).
  - Host: matrix determinant lemma
        logdet(I_576 + a V^T V) = logdet(I_96 + a V V^T)
    so only the [2,96,96] Grams leave the device; float64 Cholesky logdets
    finish the scalar loss.
"""

import numpy as np
import ml_dtypes

_STATE = {}

# -------- fixed problem geometry (hardcoded per harness contract) --------
B, CCH, H, W = 16, 32, 192, 192
NCORES = 8
TPC = B // NCORES          # timesteps per core = 2
NG = TPC * 3               # plane groups per core
OUT = 24                   # pooled spatial size
PIX = OUT * OUT            # 576
M = 96                     # feature rows (3 maps x 32 channels)
ALPHA_E = 6.0              # 576 / (96 * eps)
ALPHA_C = 18.0             # 576 / (32 * eps)

DEBUG_TAPS = False


def _build_nc():
    import concourse.bass as bass
    import concourse.tile as tile
    from concourse import bacc, mybir

    BF = mybir.dt.bfloat16
    F32 = mybir.dt.float32
    ACT = mybir.ActivationFunctionType
    OP = mybir.AluOpType

    nc = bacc.Bacc(
        "TRN2", target_bir_lowering=False, debug=False, num_devices=NCORES
    )

    x = nc.declare_dram_parameter("x", [NG, CCH, H, W], BF, isOutput=False)
    wt = nc.declare_dram_parameter("wt", [96, 288], BF, isOutput=False)
    ident = nc.declare_dram_parameter("ident", [96, 96], BF, isOutput=False)
    g_out = nc.declare_dram_parameter("g_out", [TPC, M, M], F32, isOutput=True)
    if DEBUG_TAPS:
        pooled_out = nc.declare_dram_parameter(
            "pooled_out", [NG * 32, PIX], F32, isOutput=True
        )
        v_out = nc.declare_dram_parameter("v_out", [96, TPC * PIX], F32, isOutput=True)

    with tile.TileContext(nc) as tc:
        with (
            tc.tile_pool(name="persist", bufs=1) as persist,
            tc.tile_pool(name="slab", bufs=1) as slab_pool,
            tc.tile_pool(name="red", bufs=2) as red_pool,
            tc.tile_pool(name="xrep", bufs=2) as xrep_pool,
            tc.tile_pool(name="vt", bufs=2) as vt_pool,
            tc.tile_pool(name="psc", bufs=2, space="PSUM") as psc_pool,
            tc.tile_pool(name="pst", bufs=2, space="PSUM") as pst_pool,
            tc.tile_pool(name="psg", bufs=2, space="PSUM") as psg_pool,
        ):
            # wt/id go on the scalar HWDGE ring so slab 0 is first on sync
            wt_sb = persist.tile([96, 288], BF, tag="wt")
            nc.scalar.dma_start(out=wt_sb[:], in_=wt.ap())
            id_sb = persist.tile([96, 96], BF, tag="id")
            nc.scalar.dma_start(out=id_sb[:], in_=ident.ap())
            v_sb = persist.tile([96, TPC * PIX], BF, tag="v")
            g_sb = persist.tile([96, TPC * 96], F32, tag="g")

            # ---- prefetch all 6 slabs up front: each is one contiguous
            # 2.25MB DMA, partition=(c, quarter).  Dispatching them before
            # any small DMA keeps the 8 shared HWDGE completion-semaphore
            # lanes fresh — interleaving would chain slab loads behind tiny
            # sbuf-to-sbuf DMAs from two groups earlier.
            slabs = []
            for g in range(NG):
                slab = slab_pool.tile([128, 9216], BF, tag=f"slab{g}")
                nc.sync.dma_start(
                    out=slab[:],
                    in_=x.ap()[g].rearrange("c (q h) w -> (c q) (h w)", q=4),
                )
                slabs.append(slab)

            for g in range(NG):
                t, m = divmod(g, 3)
                slab = slabs[g]

                # ---- pooling: pairwise tensor_tensor tree (bf16 2x mode) ----
                # per partition: 48 rows x 192 cols = (48h, 24x, 8w)
                sv = slab[:].rearrange("p (h x w) -> p h x w", h=48, x=24, w=8)
                t1 = red_pool.tile([128, 4608], BF, tag="t1")
                t1v = t1[:].rearrange("p (h x w) -> p h x w", h=48, x=24, w=4)
                nc.vector.tensor_tensor(
                    out=t1v, in0=sv[:, :, :, 0:4], in1=sv[:, :, :, 4:8], op=OP.add
                )
                t2 = red_pool.tile([128, 2304], BF, tag="t2")
                t2v = t2[:].rearrange("p (h x w) -> p h x w", h=48, x=24, w=2)
                nc.vector.tensor_tensor(
                    out=t2v, in0=t1v[:, :, :, 0:2], in1=t1v[:, :, :, 2:4], op=OP.add
                )
                # h-direction 8:1 before the final w-pair: (6y, 8r, 48xw)
                t2r = t2[:].rearrange("p (y r s) -> p y r s", y=6, r=8, s=48)
                t3 = red_pool.tile([128, 1152], BF, tag="t3")
                t3v = t3[:].rearrange("p (y r s) -> p y r s", y=6, r=4, s=48)
                nc.vector.tensor_tensor(
                    out=t3v, in0=t2r[:, :, 0:4, :], in1=t2r[:, :, 4:8, :], op=OP.add
                )
                t4 = red_pool.tile([128, 576], BF, tag="t4")
                t4v = t4[:].rearrange("p (y r s) -> p y r s", y=6, r=2, s=48)
                nc.vector.tensor_tensor(
                    out=t4v, in0=t3v[:, :, 0:2, :], in1=t3v[:, :, 2:4, :], op=OP.add
                )
                t5 = red_pool.tile([128, 288], BF, tag="t5")
                t5v = t5[:].rearrange("p (y r s) -> p y r s", y=6, r=1, s=48)
                nc.vector.tensor_tensor(
                    out=t5v, in0=t4v[:, :, 0:1, :], in1=t4v[:, :, 1:2, :], op=OP.add
                )
                # final w-pair written straight into the x-padded 26-wide row
                # layout: pooled [128=(c,q), (6y, 26x)], cols 1..24 are data
                t5w = t5[:].rearrange("p (y x w) -> p y x w", y=6, x=24, w=2)
                pooled = red_pool.tile([128, 6 * 26], BF, tag="pooled")
                pv26 = pooled[:].rearrange("p (y x) -> p y x", y=6, x=26)
                pv0 = pv26[:, :, 1:25].rearrange("p y (x w) -> p y x w", w=1)
                nc.vector.tensor_tensor(
                    out=pv0, in0=t5w[:, :, :, 0:1], in1=t5w[:, :, :, 1:2], op=OP.add
                )
                # x reflect pads, lane-local (cols 0,25 <- cols 2,23).
                # On the scalar engine: gpsimd crawls here (SBUF-port
                # contention with concurrent 2-port DVE ops).
                nc.scalar.copy(out=pv26[:, :, 0:1], in_=pv26[:, :, 2:3])
                nc.scalar.copy(out=pv26[:, :, 25:26], in_=pv26[:, :, 23:24])

                # ---- build xrep [96=(dy,c), 24y, 26x]: all-contiguous DMAs --
                xrep = xrep_pool.tile([96, 24 * 26], BF, tag="xrep")
                # gather quarters straight into the dy=1 block
                nc.scalar.dma_start(out=xrep[32:64, :], in_=pooled[:])
                # dy=0: row slot y holds source row y-1; reflect(-1)=row1
                nc.scalar.dma_start(
                    out=xrep[0:32, 26:624], in_=xrep[32:64, 0:598]
                )
                nc.scalar.dma_start(out=xrep[0:32, 0:26], in_=xrep[32:64, 26:52])
                # dy=2: row slot y holds source row y+1; reflect(24)=row22
                nc.scalar.dma_start(
                    out=xrep[64:96, 0:598], in_=xrep[32:64, 26:624]
                )
                nc.scalar.dma_start(
                    out=xrep[64:96, 598:624], in_=xrep[32:64, 572:598]
                )
                xr3 = xrep[:].rearrange("p (y x) -> p y x", y=OUT, x=26)

                if DEBUG_TAPS:
                    nc.gpsimd.dma_start(
                        out=pooled_out.ap()[g * 32 : (g + 1) * 32],
                        in_=xr3[32:64, :, 1:25],
                    )

                # ---- conv: 2 halves x 3 dx matmuls, K=(dy,ic)=96 ------------
                for half in range(2):
                    pc = psc_pool.tile([32, 288], F32, tag="convps")
                    for dx in range(3):
                        nc.tensor.matmul(
                            pc[:],
                            wt_sb[:, (m * 3 + dx) * 32 : (m * 3 + dx + 1) * 32],
                            xr3[:, 12 * half : 12 * half + 12, dx : dx + 24],
                            start=(dx == 0),
                            stop=(dx == 2),
                        )
                    # LeakyReLU(0.2) on the scalar engine, PSUM -> SBUF bf16
                    nc.scalar.activation(
                        out=v_sb[
                            m * 32 : (m + 1) * 32,
                            t * PIX + half * 288 : t * PIX + (half + 1) * 288,
                        ],
                        in_=pc[:],
                        func=ACT.Prelu,
                        alpha=0.2,
                    )

                # ---- Gram per t once its 3 maps are done --------------------
                if m == 2:
                    if DEBUG_TAPS:
                        nc.gpsimd.dma_start(
                            out=v_out.ap()[:, t * PIX : (t + 1) * PIX],
                            in_=v_sb[:, t * PIX : (t + 1) * PIX],
                        )
                    gp = psg_pool.tile([96, 96], F32, tag="gram")
                    for c5 in range(5):
                        sz = 128 if c5 < 4 else 64
                        vsl = v_sb[:, t * PIX + c5 * 128 : t * PIX + c5 * 128 + sz]
                        pt = pst_pool.tile([128, 96], BF, tag="vtps")
                        nc.tensor.transpose(pt[:sz, :], vsl, id_sb[:])
                        vtt = vt_pool.tile([128, 96], BF, tag="vt")
                        nc.scalar.copy(out=vtt[:sz, :], in_=pt[:sz, :])
                        nc.tensor.matmul(
                            gp[:], vtt[:sz, :], vtt[:sz, :],
                            start=(c5 == 0), stop=(c5 == 4),
                        )
                    nc.scalar.copy(out=g_sb[:, t * 96 : (t + 1) * 96], in_=gp[:])
                    nc.sync.dma_start(
                        out=g_out.ap()[t], in_=g_sb[:, t * 96 : (t + 1) * 96]
                    )

    nc.finalize()
    return nc


def _get_nc():
    if "nc" not in _STATE:
        _STATE["nc"] = _build_nc()
    return _STATE["nc"]


def _prep_weights(W1, W2, W3):
    # wt[(dy,ic), (m,dx,oc)] = W_m[oc, ic, dy, dx] / 64   (pool-mean folded in)
    w = np.stack([np.asarray(Wi, np.float64) for Wi in (W1, W2, W3)])
    wt = w.transpose(3, 2, 0, 4, 1).reshape(96, 288) / 64.0
    return wt.astype(ml_dtypes.bfloat16)


def _host_loss(G):
    G = np.asarray(G, np.float64)  # [16, 96, 96]
    T = G.shape[0]
    I96 = np.eye(M)
    Me = I96[None] + ALPHA_E * G
    ld_e = 2.0 * np.log(
        np.diagonal(np.linalg.cholesky(Me), axis1=-2, axis2=-1)
    ).sum()
    blocks = np.stack(
        [G[:, 32 * c : 32 * (c + 1), 32 * c : 32 * (c + 1)] for c in range(3)]
    )  # [3, T, 32, 32]
    Mc = np.eye(32)[None, None] + ALPHA_C * blocks
    ld_c = 2.0 * np.log(
        np.diagonal(np.linalg.cholesky(Mc), axis1=-2, axis2=-1)
    ).sum()
    loss_expd = ld_e / (2.0 * T)
    loss_comp = (32.0 / M) * ld_c / (2.0 * T)
    return np.float32(loss_expd - loss_comp)


def run_device(inputs, **kw):
    """Run the bass kernel; returns (G [16,96,96], BassKernelResults)."""
    from concourse.bass_utils import run_bass_kernel_spmd

    nc = _get_nc()
    wt = _prep_weights(inputs["W1"], inputs["W2"], inputs["W3"])
    ident = np.eye(96, dtype=ml_dtypes.bfloat16)
    ms = np.asarray(inputs["ms_fea"], np.float32)
    pan = np.asarray(inputs["pan_fea"], np.float32)
    alf = np.asarray(inputs["all_fea"], np.float32)
    in_maps = []
    for i in range(NCORES):
        sl = slice(TPC * i, TPC * (i + 1))
        # x[t*3+m] = (ms,pan,alf)[m][t]
        xs = np.stack([ms[sl], pan[sl], alf[sl]], axis=1).reshape(
            NG, CCH, H, W
        )
        in_maps.append(
            {"x": xs.astype(ml_dtypes.bfloat16), "wt": wt, "ident": ident}
        )
    res = run_bass_kernel_spmd(nc, in_maps, core_ids=list(range(NCORES)), **kw)
    G = np.concatenate([np.asarray(r["g_out"]) for r in res.results], axis=0)
    return G, res


def kernel(**inputs):
    G, _ = run_device(inputs)
    return _host_loss(G)


# revision 14
# speedup vs baseline: 2.0765x; 1.0352x over previous
"""MCR loss kernel for Trainium2 (8 NeuronCores).

Strategy:
  - Shard batch T=16 -> 2 timesteps per core (data parallel, no collectives).
  - Host converts inputs to bf16: halves HBM traffic (the roofline term) and
    enables the DVE 2x packed mode for the pooling adds.
  - Per core, 6 plane-groups (2 timesteps x 3 maps).  Each group's 32x192x192
    plane stack is one contiguous 2.25MB DMA into [128=(c,quarter), 9216].
  - 8x8 avg-pool (as sum; 1/64 folded into conv weights) via a 6-op
    tensor_tensor pairwise tree on the vector engine (2x mode on bf16).
  - Reflect-pad + dy-replication built by small SBUF->SBUF DMAs on the
    scalar-engine HWDGE ring; 3x3 conv as 3 PE matmuls with K=(dy,ic)=96;
    LeakyReLU(0.2) natively on the scalar engine (Lrelu, PSUM->SBUF).
  - Gram G_t = V_t V_t^T via PE transpose + bf16 matmul (f32 PSUM accum).
  - Host: matrix determinant lemma
        logdet(I_576 + a V^T V) = logdet(I_96 + a V V^T)
    so only the [2,96,96] Grams leave the device; float64 Cholesky logdets
    finish the scalar loss.
"""

import numpy as np
import ml_dtypes

_STATE = {}

# -------- fixed problem geometry (hardcoded per harness contract) --------
B, CCH, H, W = 16, 32, 192, 192
NCORES = 8
TPC = B // NCORES          # timesteps per core = 2
NG = TPC * 3               # plane groups per core
OUT = 24                   # pooled spatial size
PIX = OUT * OUT            # 576
M = 96                     # feature rows (3 maps x 32 channels)
ALPHA_E = 6.0              # 576 / (96 * eps)
ALPHA_C = 18.0             # 576 / (32 * eps)

DEBUG_TAPS = False


def _build_nc():
    import concourse.bass as bass
    import concourse.tile as tile
    from concourse import bacc, mybir

    BF = mybir.dt.bfloat16
    F32 = mybir.dt.float32
    ACT = mybir.ActivationFunctionType
    OP = mybir.AluOpType

    nc = bacc.Bacc(
        "TRN2", target_bir_lowering=False, debug=False, num_devices=NCORES
    )

    x = nc.declare_dram_parameter("x", [NG, CCH, H, W], BF, isOutput=False)
    wt = nc.declare_dram_parameter("wt", [96, 288], BF, isOutput=False)
    ident = nc.declare_dram_parameter("ident", [96, 96], BF, isOutput=False)
    g_out = nc.declare_dram_parameter("g_out", [TPC, M, M], F32, isOutput=True)
    if DEBUG_TAPS:
        pooled_out = nc.declare_dram_parameter(
            "pooled_out", [NG * 32, PIX], F32, isOutput=True
        )
        v_out = nc.declare_dram_parameter("v_out", [96, TPC * PIX], F32, isOutput=True)

    with tile.TileContext(nc) as tc:
        with (
            tc.tile_pool(name="persist", bufs=1) as persist,
            tc.tile_pool(name="slab", bufs=1) as slab_pool,
            tc.tile_pool(name="red", bufs=2) as red_pool,
            tc.tile_pool(name="xrep", bufs=2) as xrep_pool,
            tc.tile_pool(name="vt", bufs=2) as vt_pool,
            tc.tile_pool(name="psc", bufs=2, space="PSUM") as psc_pool,
            tc.tile_pool(name="pst", bufs=2, space="PSUM") as pst_pool,
            tc.tile_pool(name="psg", bufs=2, space="PSUM") as psg_pool,
        ):
            # wt/id go on the scalar HWDGE ring so slab 0 is first on sync
            wt_sb = persist.tile([96, 288], BF, tag="wt")
            nc.gpsimd.dma_start(out=wt_sb[:], in_=wt.ap())
            id_sb = persist.tile([96, 96], BF, tag="id")
            nc.gpsimd.dma_start(out=id_sb[:], in_=ident.ap())
            v_sb = persist.tile([96, TPC * PIX], BF, tag="v")
            g_sb = persist.tile([96, TPC * 96], F32, tag="g")

            # ---- prefetch all 6 slabs up front: each is one contiguous
            # 2.25MB DMA, partition=(c, quarter).  Dispatching them before
            # any small DMA keeps the 8 shared HWDGE completion-semaphore
            # lanes fresh — interleaving would chain slab loads behind tiny
            # sbuf-to-sbuf DMAs from two groups earlier.
            slabs = []
            for g in range(NG):
                slab = slab_pool.tile([128, 9216], BF, tag=f"slab{g}")
                nc.sync.dma_start(
                    out=slab[:],
                    in_=x.ap()[g].rearrange("c (q h) w -> (c q) (h w)", q=4),
                )
                slabs.append(slab)

            for g in range(NG):
                t, m = divmod(g, 3)
                slab = slabs[g]

                # ---- pooling: pairwise tensor_tensor tree (bf16 2x mode) ----
                # per partition: 48 rows x 192 cols = (48h, 24x, 8w)
                sv = slab[:].rearrange("p (h x w) -> p h x w", h=48, x=24, w=8)
                t1 = red_pool.tile([128, 4608], BF, tag="t1")
                t1v = t1[:].rearrange("p (h x w) -> p h x w", h=48, x=24, w=4)
                nc.vector.tensor_tensor(
                    out=t1v, in0=sv[:, :, :, 0:4], in1=sv[:, :, :, 4:8], op=OP.add
                )
                t2 = red_pool.tile([128, 2304], BF, tag="t2")
                t2v = t2[:].rearrange("p (h x w) -> p h x w", h=48, x=24, w=2)
                nc.vector.tensor_tensor(
                    out=t2v, in0=t1v[:, :, :, 0:2], in1=t1v[:, :, :, 2:4], op=OP.add
                )
                # h-direction 8:1 before the final w-pair: (6y, 8r, 48xw)
                t2r = t2[:].rearrange("p (y r s) -> p y r s", y=6, r=8, s=48)
                t3 = red_pool.tile([128, 1152], BF, tag="t3")
                t3v = t3[:].rearrange("p (y r s) -> p y r s", y=6, r=4, s=48)
                nc.vector.tensor_tensor(
                    out=t3v, in0=t2r[:, :, 0:4, :], in1=t2r[:, :, 4:8, :], op=OP.add
                )
                t4 = red_pool.tile([128, 576], BF, tag="t4")
                t4v = t4[:].rearrange("p (y r s) -> p y r s", y=6, r=2, s=48)
                nc.vector.tensor_tensor(
                    out=t4v, in0=t3v[:, :, 0:2, :], in1=t3v[:, :, 2:4, :], op=OP.add
                )
                t5 = red_pool.tile([128, 288], BF, tag="t5")
                t5v = t5[:].rearrange("p (y r s) -> p y r s", y=6, r=1, s=48)
                nc.vector.tensor_tensor(
                    out=t5v, in0=t4v[:, :, 0:1, :], in1=t4v[:, :, 1:2, :], op=OP.add
                )
                # final w-pair written straight into the x-padded 26-wide row
                # layout: pooled [128=(c,q), (6y, 26x)], cols 1..24 are data
                t5w = t5[:].rearrange("p (y x w) -> p y x w", y=6, x=24, w=2)
                pooled = red_pool.tile([128, 6 * 26], BF, tag="pooled")
                pv26 = pooled[:].rearrange("p (y x) -> p y x", y=6, x=26)
                pv0 = pv26[:, :, 1:25].rearrange("p y (x w) -> p y x w", w=1)
                nc.vector.tensor_tensor(
                    out=pv0, in0=t5w[:, :, :, 0:1], in1=t5w[:, :, :, 1:2], op=OP.add
                )
                # x reflect pads, lane-local (cols 0,25 <- cols 2,23).
                # On the scalar engine: gpsimd crawls here (SBUF-port
                # contention with concurrent 2-port DVE ops).
                nc.scalar.copy(out=pv26[:, :, 0:1], in_=pv26[:, :, 2:3])
                nc.scalar.copy(out=pv26[:, :, 25:26], in_=pv26[:, :, 23:24])

                # ---- build xrep [96=(dy,c), 24y, 26x]: all-contiguous DMAs --
                xrep = xrep_pool.tile([96, 24 * 26], BF, tag="xrep")
                # gather quarters straight into the dy=1 block
                nc.gpsimd.dma_start(out=xrep[32:64, :], in_=pooled[:])
                # dy=0: row slot y holds source row y-1; reflect(-1)=row1
                nc.gpsimd.dma_start(
                    out=xrep[0:32, 26:624], in_=xrep[32:64, 0:598]
                )
                nc.gpsimd.dma_start(out=xrep[0:32, 0:26], in_=xrep[32:64, 26:52])
                # dy=2: row slot y holds source row y+1; reflect(24)=row22
                nc.gpsimd.dma_start(
                    out=xrep[64:96, 0:598], in_=xrep[32:64, 26:624]
                )
                nc.gpsimd.dma_start(
                    out=xrep[64:96, 598:624], in_=xrep[32:64, 572:598]
                )
                xr3 = xrep[:].rearrange("p (y x) -> p y x", y=OUT, x=26)

                if DEBUG_TAPS:
                    nc.gpsimd.dma_start(
                        out=pooled_out.ap()[g * 32 : (g + 1) * 32],
                        in_=xr3[32:64, :, 1:25],
                    )

                # ---- conv: 2 halves x 3 dx matmuls, K=(dy,ic)=96 ------------
                for half in range(2):
                    pc = psc_pool.tile([32, 288], F32, tag="convps")
                    for dx in range(3):
                        nc.tensor.matmul(
                            pc[:],
                            wt_sb[:, (m * 3 + dx) * 32 : (m * 3 + dx + 1) * 32],
                            xr3[:, 12 * half : 12 * half + 12, dx : dx + 24],
                            start=(dx == 0),
                            stop=(dx == 2),
                        )
                    # LeakyReLU(0.2) on the scalar engine, PSUM -> SBUF bf16
                    nc.scalar.activation(
                        out=v_sb[
                            m * 32 : (m + 1) * 32,
                            t * PIX + half * 288 : t * PIX + (half + 1) * 288,
                        ],
                        in_=pc[:],
                        func=ACT.Prelu,
                        alpha=0.2,
                    )

                # ---- Gram per t once its 3 maps are done --------------------
                if m == 2:
                    if DEBUG_TAPS:
                        nc.gpsimd.dma_start(
                            out=v_out.ap()[:, t * PIX : (t + 1) * PIX],
                            in_=v_sb[:, t * PIX : (t + 1) * PIX],
                        )
                    gp = psg_pool.tile([96, 96], F32, tag="gram")
                    for c5 in range(5):
                        sz = 128 if c5 < 4 else 64
                        vsl = v_sb[:, t * PIX + c5 * 128 : t * PIX + c5 * 128 + sz]
                        pt = pst_pool.tile([128, 96], BF, tag="vtps")
                        nc.tensor.transpose(pt[:sz, :], vsl, id_sb[:])
                        vtt = vt_pool.tile([128, 96], BF, tag="vt")
                        nc.scalar.copy(out=vtt[:sz, :], in_=pt[:sz, :])
                        nc.tensor.matmul(
                            gp[:], vtt[:sz, :], vtt[:sz, :],
                            start=(c5 == 0), stop=(c5 == 4),
                        )
                    nc.scalar.copy(out=g_sb[:, t * 96 : (t + 1) * 96], in_=gp[:])
                    nc.sync.dma_start(
                        out=g_out.ap()[t], in_=g_sb[:, t * 96 : (t + 1) * 96]
                    )

    nc.finalize()
    return nc


def _get_nc():
    if "nc" not in _STATE:
        _STATE["nc"] = _build_nc()
    return _STATE["nc"]


def _prep_weights(W1, W2, W3):
    # wt[(dy,ic), (m,dx,oc)] = W_m[oc, ic, dy, dx] / 64   (pool-mean folded in)
    w = np.stack([np.asarray(Wi, np.float64) for Wi in (W1, W2, W3)])
    wt = w.transpose(3, 2, 0, 4, 1).reshape(96, 288) / 64.0
    return wt.astype(ml_dtypes.bfloat16)


def _host_loss(G):
    G = np.asarray(G, np.float64)  # [16, 96, 96]
    T = G.shape[0]
    I96 = np.eye(M)
    Me = I96[None] + ALPHA_E * G
    ld_e = 2.0 * np.log(
        np.diagonal(np.linalg.cholesky(Me), axis1=-2, axis2=-1)
    ).sum()
    blocks = np.stack(
        [G[:, 32 * c : 32 * (c + 1), 32 * c : 32 * (c + 1)] for c in range(3)]
    )  # [3, T, 32, 32]
    Mc = np.eye(32)[None, None] + ALPHA_C * blocks
    ld_c = 2.0 * np.log(
        np.diagonal(np.linalg.cholesky(Mc), axis1=-2, axis2=-1)
    ).sum()
    loss_expd = ld_e / (2.0 * T)
    loss_comp = (32.0 / M) * ld_c / (2.0 * T)
    return np.float32(loss_expd - loss_comp)


def run_device(inputs, **kw):
    """Run the bass kernel; returns (G [16,96,96], BassKernelResults)."""
    from concourse.bass_utils import run_bass_kernel_spmd

    nc = _get_nc()
    wt = _prep_weights(inputs["W1"], inputs["W2"], inputs["W3"])
    ident = np.eye(96, dtype=ml_dtypes.bfloat16)
    ms = np.asarray(inputs["ms_fea"], np.float32)
    pan = np.asarray(inputs["pan_fea"], np.float32)
    alf = np.asarray(inputs["all_fea"], np.float32)
    in_maps = []
    for i in range(NCORES):
        sl = slice(TPC * i, TPC * (i + 1))
        # x[t*3+m] = (ms,pan,alf)[m][t]
        xs = np.stack([ms[sl], pan[sl], alf[sl]], axis=1).reshape(
            NG, CCH, H, W
        )
        in_maps.append(
            {"x": xs.astype(ml_dtypes.bfloat16), "wt": wt, "ident": ident}
        )
    res = run_bass_kernel_spmd(nc, in_maps, core_ids=list(range(NCORES)), **kw)
    G = np.concatenate([np.asarray(r["g_out"]) for r in res.results], axis=0)
    return G, res


def kernel(**inputs):
    G, _ = run_device(inputs)
    return _host_loss(G)


# revision 15
# speedup vs baseline: 2.2506x; 1.0839x over previous
"""MCR loss kernel for Trainium2 (8 NeuronCores).

Strategy:
  - Shard batch T=16 -> 2 timesteps per core (data parallel, no collectives).
  - Host converts inputs to bf16: halves HBM traffic (the roofline term) and
    enables the DVE 2x packed mode for the pooling adds.
  - Per core, 6 plane-groups (2 timesteps x 3 maps).  Each group's 32x192x192
    plane stack is one contiguous 2.25MB DMA into [128=(c,quarter), 9216].
  - 8x8 avg-pool (as sum; 1/64 folded into conv weights) via a 6-op
    tensor_tensor pairwise tree on the vector engine (2x mode on bf16).
  - Reflect-pad + dy-replication built by small SBUF->SBUF DMAs on the
    scalar-engine HWDGE ring; 3x3 conv as 3 PE matmuls with K=(dy,ic)=96;
    LeakyReLU(0.2) natively on the scalar engine (Lrelu, PSUM->SBUF).
  - Gram G_t = V_t V_t^T via PE transpose + bf16 matmul (f32 PSUM accum).
  - Host: matrix determinant lemma
        logdet(I_576 + a V^T V) = logdet(I_96 + a V V^T)
    so only the [2,96,96] Grams leave the device; float64 Cholesky logdets
    finish the scalar loss.
"""

import numpy as np
import ml_dtypes

_STATE = {}

# -------- fixed problem geometry (hardcoded per harness contract) --------
B, CCH, H, W = 16, 32, 192, 192
NCORES = 8
TPC = B // NCORES          # timesteps per core = 2
NG = TPC * 3               # plane groups per core
OUT = 24                   # pooled spatial size
PIX = OUT * OUT            # 576
M = 96                     # feature rows (3 maps x 32 channels)
ALPHA_E = 6.0              # 576 / (96 * eps)
ALPHA_C = 18.0             # 576 / (32 * eps)

DEBUG_TAPS = False


def _build_nc():
    import concourse.bass as bass
    import concourse.tile as tile
    from concourse import bacc, mybir

    BF = mybir.dt.bfloat16
    F8 = mybir.dt.float8e4
    F32 = mybir.dt.float32
    ACT = mybir.ActivationFunctionType
    OP = mybir.AluOpType

    nc = bacc.Bacc(
        "TRN2", target_bir_lowering=False, debug=False, num_devices=NCORES
    )

    x = nc.declare_dram_parameter("x", [NG, CCH, H, W], F8, isOutput=False)
    wt = nc.declare_dram_parameter("wt", [96, 288], BF, isOutput=False)
    ident = nc.declare_dram_parameter("ident", [96, 96], BF, isOutput=False)
    g_out = nc.declare_dram_parameter("g_out", [TPC, M, M], F32, isOutput=True)
    if DEBUG_TAPS:
        pooled_out = nc.declare_dram_parameter(
            "pooled_out", [NG * 32, PIX], F32, isOutput=True
        )
        v_out = nc.declare_dram_parameter("v_out", [96, TPC * PIX], F32, isOutput=True)

    with tile.TileContext(nc) as tc:
        with (
            tc.tile_pool(name="persist", bufs=1) as persist,
            tc.tile_pool(name="slab", bufs=1) as slab_pool,
            tc.tile_pool(name="red", bufs=2) as red_pool,
            tc.tile_pool(name="xrep", bufs=2) as xrep_pool,
            tc.tile_pool(name="vt", bufs=2) as vt_pool,
            tc.tile_pool(name="psc", bufs=2, space="PSUM") as psc_pool,
            tc.tile_pool(name="pst", bufs=2, space="PSUM") as pst_pool,
            tc.tile_pool(name="psg", bufs=2, space="PSUM") as psg_pool,
        ):
            # wt/id go on the scalar HWDGE ring so slab 0 is first on sync
            wt_sb = persist.tile([96, 288], BF, tag="wt")
            nc.sync.dma_start(out=wt_sb[:], in_=wt.ap())
            id_sb = persist.tile([96, 96], BF, tag="id")
            nc.sync.dma_start(out=id_sb[:], in_=ident.ap())
            v_sb = persist.tile([96, TPC * PIX], BF, tag="v")
            g_sb = persist.tile([96, TPC * 96], F32, tag="g")

            # ---- prefetch all 6 slabs up front: each is one contiguous
            # 2.25MB DMA, partition=(c, quarter).  Dispatching them before
            # any small DMA keeps the 8 shared HWDGE completion-semaphore
            # lanes fresh — interleaving would chain slab loads behind tiny
            # sbuf-to-sbuf DMAs from two groups earlier.
            slabs = []
            for g in range(NG):
                slab = slab_pool.tile([128, 9216], BF, tag=f"slab{g}")
                # SWDGE cast-DMA: reads fp8 from HBM, writes bf16 to SBUF
                nc.gpsimd.dma_start(
                    out=slab[:],
                    in_=x.ap()[g].rearrange("c (q h) w -> (c q) (h w)", q=4),
                )
                slabs.append(slab)

            for g in range(NG):
                t, m = divmod(g, 3)
                slab = slabs[g]

                # ---- pooling: pairwise tensor_tensor tree (bf16 2x mode) ----
                # per partition: 48 rows x 192 cols = (48h, 24x, 8w)
                sv = slab[:].rearrange("p (h x w) -> p h x w", h=48, x=24, w=8)
                t1 = red_pool.tile([128, 4608], BF, tag="t1")
                t1v = t1[:].rearrange("p (h x w) -> p h x w", h=48, x=24, w=4)
                nc.vector.tensor_tensor(
                    out=t1v, in0=sv[:, :, :, 0:4], in1=sv[:, :, :, 4:8], op=OP.add
                )
                t2 = red_pool.tile([128, 2304], BF, tag="t2")
                t2v = t2[:].rearrange("p (h x w) -> p h x w", h=48, x=24, w=2)
                nc.vector.tensor_tensor(
                    out=t2v, in0=t1v[:, :, :, 0:2], in1=t1v[:, :, :, 2:4], op=OP.add
                )
                # h-direction 8:1 before the final w-pair: (6y, 8r, 48xw)
                t2r = t2[:].rearrange("p (y r s) -> p y r s", y=6, r=8, s=48)
                t3 = red_pool.tile([128, 1152], BF, tag="t3")
                t3v = t3[:].rearrange("p (y r s) -> p y r s", y=6, r=4, s=48)
                nc.vector.tensor_tensor(
                    out=t3v, in0=t2r[:, :, 0:4, :], in1=t2r[:, :, 4:8, :], op=OP.add
                )
                t4 = red_pool.tile([128, 576], BF, tag="t4")
                t4v = t4[:].rearrange("p (y r s) -> p y r s", y=6, r=2, s=48)
                nc.vector.tensor_tensor(
                    out=t4v, in0=t3v[:, :, 0:2, :], in1=t3v[:, :, 2:4, :], op=OP.add
                )
                t5 = red_pool.tile([128, 288], BF, tag="t5")
                t5v = t5[:].rearrange("p (y r s) -> p y r s", y=6, r=1, s=48)
                nc.vector.tensor_tensor(
                    out=t5v, in0=t4v[:, :, 0:1, :], in1=t4v[:, :, 1:2, :], op=OP.add
                )
                # final w-pair written straight into the x-padded 26-wide row
                # layout: pooled [128=(c,q), (6y, 26x)], cols 1..24 are data
                t5w = t5[:].rearrange("p (y x w) -> p y x w", y=6, x=24, w=2)
                pooled = red_pool.tile([128, 6 * 26], BF, tag="pooled")
                pv26 = pooled[:].rearrange("p (y x) -> p y x", y=6, x=26)
                pv0 = pv26[:, :, 1:25].rearrange("p y (x w) -> p y x w", w=1)
                nc.vector.tensor_tensor(
                    out=pv0, in0=t5w[:, :, :, 0:1], in1=t5w[:, :, :, 1:2], op=OP.add
                )
                # x reflect pads, lane-local (cols 0,25 <- cols 2,23).
                # On the scalar engine: gpsimd crawls here (SBUF-port
                # contention with concurrent 2-port DVE ops).
                nc.scalar.copy(out=pv26[:, :, 0:1], in_=pv26[:, :, 2:3])
                nc.scalar.copy(out=pv26[:, :, 25:26], in_=pv26[:, :, 23:24])

                # ---- build xrep [96=(dy,c), 24y, 26x]: all-contiguous DMAs --
                xrep = xrep_pool.tile([96, 24 * 26], BF, tag="xrep")
                # gather quarters straight into the dy=1 block
                nc.gpsimd.dma_start(out=xrep[32:64, :], in_=pooled[:])
                # dy=0: row slot y holds source row y-1; reflect(-1)=row1
                nc.gpsimd.dma_start(
                    out=xrep[0:32, 26:624], in_=xrep[32:64, 0:598]
                )
                nc.gpsimd.dma_start(out=xrep[0:32, 0:26], in_=xrep[32:64, 26:52])
                # dy=2: row slot y holds source row y+1; reflect(24)=row22
                nc.gpsimd.dma_start(
                    out=xrep[64:96, 0:598], in_=xrep[32:64, 26:624]
                )
                nc.gpsimd.dma_start(
                    out=xrep[64:96, 598:624], in_=xrep[32:64, 572:598]
                )
                xr3 = xrep[:].rearrange("p (y x) -> p y x", y=OUT, x=26)

                if DEBUG_TAPS:
                    nc.gpsimd.dma_start(
                        out=pooled_out.ap()[g * 32 : (g + 1) * 32],
                        in_=xr3[32:64, :, 1:25],
                    )

                # ---- conv: 2 halves x 3 dx matmuls, K=(dy,ic)=96 ------------
                for half in range(2):
                    pc = psc_pool.tile([32, 288], F32, tag="convps")
                    for dx in range(3):
                        nc.tensor.matmul(
                            pc[:],
                            wt_sb[:, (m * 3 + dx) * 32 : (m * 3 + dx + 1) * 32],
                            xr3[:, 12 * half : 12 * half + 12, dx : dx + 24],
                            start=(dx == 0),
                            stop=(dx == 2),
                        )
                    # LeakyReLU(0.2) on the scalar engine, PSUM -> SBUF bf16
                    nc.scalar.activation(
                        out=v_sb[
                            m * 32 : (m + 1) * 32,
                            t * PIX + half * 288 : t * PIX + (half + 1) * 288,
                        ],
                        in_=pc[:],
                        func=ACT.Prelu,
                        alpha=0.2,
                    )

                # ---- Gram per t once its 3 maps are done --------------------
                if m == 2:
                    if DEBUG_TAPS:
                        nc.gpsimd.dma_start(
                            out=v_out.ap()[:, t * PIX : (t + 1) * PIX],
                            in_=v_sb[:, t * PIX : (t + 1) * PIX],
                        )
                    gp = psg_pool.tile([96, 96], F32, tag="gram")
                    for c5 in range(5):
                        sz = 128 if c5 < 4 else 64
                        vsl = v_sb[:, t * PIX + c5 * 128 : t * PIX + c5 * 128 + sz]
                        pt = pst_pool.tile([128, 96], BF, tag="vtps")
                        nc.tensor.transpose(pt[:sz, :], vsl, id_sb[:])
                        vtt = vt_pool.tile([128, 96], BF, tag="vt")
                        nc.scalar.copy(out=vtt[:sz, :], in_=pt[:sz, :])
                        nc.tensor.matmul(
                            gp[:], vtt[:sz, :], vtt[:sz, :],
                            start=(c5 == 0), stop=(c5 == 4),
                        )
                    nc.scalar.copy(out=g_sb[:, t * 96 : (t + 1) * 96], in_=gp[:])
                    nc.sync.dma_start(
                        out=g_out.ap()[t], in_=g_sb[:, t * 96 : (t + 1) * 96]
                    )

    nc.finalize()
    return nc


def _get_nc():
    if "nc" not in _STATE:
        _STATE["nc"] = _build_nc()
    return _STATE["nc"]


def _prep_weights(W1, W2, W3):
    # wt[(dy,ic), (m,dx,oc)] = W_m[oc, ic, dy, dx] / 64   (pool-mean folded in)
    w = np.stack([np.asarray(Wi, np.float64) for Wi in (W1, W2, W3)])
    wt = w.transpose(3, 2, 0, 4, 1).reshape(96, 288) / 64.0
    return wt.astype(ml_dtypes.bfloat16)


def _host_loss(G):
    G = np.asarray(G, np.float64)  # [16, 96, 96]
    T = G.shape[0]
    I96 = np.eye(M)
    Me = I96[None] + ALPHA_E * G
    ld_e = 2.0 * np.log(
        np.diagonal(np.linalg.cholesky(Me), axis1=-2, axis2=-1)
    ).sum()
    blocks = np.stack(
        [G[:, 32 * c : 32 * (c + 1), 32 * c : 32 * (c + 1)] for c in range(3)]
    )  # [3, T, 32, 32]
    Mc = np.eye(32)[None, None] + ALPHA_C * blocks
    ld_c = 2.0 * np.log(
        np.diagonal(np.linalg.cholesky(Mc), axis1=-2, axis2=-1)
    ).sum()
    loss_expd = ld_e / (2.0 * T)
    loss_comp = (32.0 / M) * ld_c / (2.0 * T)
    return np.float32(loss_expd - loss_comp)


def run_device(inputs, **kw):
    """Run the bass kernel; returns (G [16,96,96], BassKernelResults)."""
    from concourse.bass_utils import run_bass_kernel_spmd

    nc = _get_nc()
    wt = _prep_weights(inputs["W1"], inputs["W2"], inputs["W3"])
    ident = np.eye(96, dtype=ml_dtypes.bfloat16)
    ms = np.asarray(inputs["ms_fea"], np.float32)
    pan = np.asarray(inputs["pan_fea"], np.float32)
    alf = np.asarray(inputs["all_fea"], np.float32)
    in_maps = []
    for i in range(NCORES):
        sl = slice(TPC * i, TPC * (i + 1))
        # x[t*3+m] = (ms,pan,alf)[m][t]
        xs = np.stack([ms[sl], pan[sl], alf[sl]], axis=1).reshape(
            NG, CCH, H, W
        )
        in_maps.append(
            {"x": xs.astype(ml_dtypes.float8_e4m3fn), "wt": wt, "ident": ident}
        )
    res = run_bass_kernel_spmd(nc, in_maps, core_ids=list(range(NCORES)), **kw)
    G = np.concatenate([np.asarray(r["g_out"]) for r in res.results], axis=0)
    return G, res


def kernel(**inputs):
    G, _ = run_device(inputs)
    return _host_loss(G)


# revision 18
# speedup vs baseline: 2.2657x; 1.0067x over previous
"""MCR loss kernel for Trainium2 (8 NeuronCores).

Strategy:
  - Shard batch T=16 -> 2 timesteps per core (data parallel, no collectives).
  - Host converts inputs to bf16: halves HBM traffic (the roofline term) and
    enables the DVE 2x packed mode for the pooling adds.
  - Per core, 6 plane-groups (2 timesteps x 3 maps).  Each group's 32x192x192
    plane stack is one contiguous 2.25MB DMA into [128=(c,quarter), 9216].
  - 8x8 avg-pool (as sum; 1/64 folded into conv weights) via a 6-op
    tensor_tensor pairwise tree on the vector engine (2x mode on bf16).
  - Reflect-pad + dy-replication built by small SBUF->SBUF DMAs on the
    scalar-engine HWDGE ring; 3x3 conv as 3 PE matmuls with K=(dy,ic)=96;
    LeakyReLU(0.2) natively on the scalar engine (Lrelu, PSUM->SBUF).
  - Gram G_t = V_t V_t^T via PE transpose + bf16 matmul (f32 PSUM accum).
  - Host: matrix determinant lemma
        logdet(I_576 + a V^T V) = logdet(I_96 + a V V^T)
    so only the [2,96,96] Grams leave the device; float64 Cholesky logdets
    finish the scalar loss.
"""

import numpy as np
import ml_dtypes

_STATE = {}

# -------- fixed problem geometry (hardcoded per harness contract) --------
B, CCH, H, W = 16, 32, 192, 192
NCORES = 8
TPC = B // NCORES          # timesteps per core = 2
NG = TPC * 3               # plane groups per core
OUT = 24                   # pooled spatial size
PIX = OUT * OUT            # 576
M = 96                     # feature rows (3 maps x 32 channels)
ALPHA_E = 6.0              # 576 / (96 * eps)
ALPHA_C = 18.0             # 576 / (32 * eps)

DEBUG_TAPS = False


def _build_nc():
    import concourse.bass as bass
    import concourse.tile as tile
    from concourse import bacc, mybir

    BF = mybir.dt.bfloat16
    F8 = mybir.dt.float8e4
    F32 = mybir.dt.float32
    ACT = mybir.ActivationFunctionType
    OP = mybir.AluOpType

    nc = bacc.Bacc(
        "TRN2", target_bir_lowering=False, debug=False, num_devices=NCORES
    )

    x = nc.declare_dram_parameter("x", [NG, CCH, H, W], F8, isOutput=False)
    wt = nc.declare_dram_parameter("wt", [96, 288], BF, isOutput=False)
    ident = nc.declare_dram_parameter("ident", [96, 96], BF, isOutput=False)
    g_out = nc.declare_dram_parameter("g_out", [TPC, M, M], F32, isOutput=True)
    if DEBUG_TAPS:
        pooled_out = nc.declare_dram_parameter(
            "pooled_out", [NG * 32, PIX], F32, isOutput=True
        )
        v_out = nc.declare_dram_parameter("v_out", [96, TPC * PIX], F32, isOutput=True)

    with tile.TileContext(nc) as tc:
        with (
            tc.tile_pool(name="persist", bufs=1) as persist,
            tc.tile_pool(name="slab", bufs=1) as slab_pool,
            tc.tile_pool(name="red", bufs=2) as red_pool,
            tc.tile_pool(name="xrep", bufs=2) as xrep_pool,
            tc.tile_pool(name="vt", bufs=2) as vt_pool,
            tc.tile_pool(name="psc", bufs=2, space="PSUM") as psc_pool,
            tc.tile_pool(name="pst", bufs=2, space="PSUM") as pst_pool,
            tc.tile_pool(name="psg", bufs=2, space="PSUM") as psg_pool,
        ):
            # wt/id go on the scalar HWDGE ring so slab 0 is first on sync
            wt_sb = persist.tile([96, 288], BF, tag="wt")
            nc.sync.dma_start(out=wt_sb[:], in_=wt.ap())
            id_sb = persist.tile([96, 96], BF, tag="id")
            nc.sync.dma_start(out=id_sb[:], in_=ident.ap())
            v_sb = persist.tile([96, TPC * PIX], BF, tag="v")
            g_sb = persist.tile([96, TPC * 96], F32, tag="g")

            # ---- prefetch all 6 slabs up front: each is one contiguous
            # 2.25MB DMA, partition=(c, quarter).  Dispatching them before
            # any small DMA keeps the 8 shared HWDGE completion-semaphore
            # lanes fresh — interleaving would chain slab loads behind tiny
            # sbuf-to-sbuf DMAs from two groups earlier.
            slabs = []
            for g in range(NG):
                slab = slab_pool.tile([128, 9216], BF, tag=f"slab{g}")
                # SWDGE cast-DMA: reads fp8 from HBM, writes bf16 to SBUF
                nc.gpsimd.dma_start(
                    out=slab[:],
                    in_=x.ap()[g].rearrange("c (q h) w -> (c q) (h w)", q=4),
                )
                slabs.append(slab)

            for g in range(NG):
                t, m = divmod(g, 3)
                slab = slabs[g]

                # ---- pooling: pairwise tensor_tensor tree (bf16 2x mode) ----
                # per partition: 48 rows x 192 cols = (48h, 24x, 8w)
                sv = slab[:].rearrange("p (h x w) -> p h x w", h=48, x=24, w=8)
                t1 = red_pool.tile([128, 4608], BF, tag="t1")
                t1v = t1[:].rearrange("p (h x w) -> p h x w", h=48, x=24, w=4)
                nc.vector.tensor_tensor(
                    out=t1v, in0=sv[:, :, :, 0:4], in1=sv[:, :, :, 4:8], op=OP.add
                )
                t2 = red_pool.tile([128, 2304], BF, tag="t2")
                t2v = t2[:].rearrange("p (h x w) -> p h x w", h=48, x=24, w=2)
                nc.vector.tensor_tensor(
                    out=t2v, in0=t1v[:, :, :, 0:2], in1=t1v[:, :, :, 2:4], op=OP.add
                )
                # h-direction 8:1 before the final w-pair: (6y, 8r, 48xw)
                t2r = t2[:].rearrange("p (y r s) -> p y r s", y=6, r=8, s=48)
                t3 = red_pool.tile([128, 1152], BF, tag="t3")
                t3v = t3[:].rearrange("p (y r s) -> p y r s", y=6, r=4, s=48)
                nc.vector.tensor_tensor(
                    out=t3v, in0=t2r[:, :, 0:4, :], in1=t2r[:, :, 4:8, :], op=OP.add
                )
                t4 = red_pool.tile([128, 576], BF, tag="t4")
                t4v = t4[:].rearrange("p (y r s) -> p y r s", y=6, r=2, s=48)
                nc.vector.tensor_tensor(
                    out=t4v, in0=t3v[:, :, 0:2, :], in1=t3v[:, :, 2:4, :], op=OP.add
                )
                t5 = red_pool.tile([128, 288], BF, tag="t5")
                t5v = t5[:].rearrange("p (y r s) -> p y r s", y=6, r=1, s=48)
                nc.vector.tensor_tensor(
                    out=t5v, in0=t4v[:, :, 0:1, :], in1=t4v[:, :, 1:2, :], op=OP.add
                )
                # final w-pair written straight into the x-padded 26-wide row
                # layout: pooled [128=(c,q), (6y, 26x)], cols 1..24 are data
                t5w = t5[:].rearrange("p (y x w) -> p y x w", y=6, x=24, w=2)
                pooled = red_pool.tile([128, 6 * 26], BF, tag="pooled")
                pv26 = pooled[:].rearrange("p (y x) -> p y x", y=6, x=26)
                pv0 = pv26[:, :, 1:25].rearrange("p y (x w) -> p y x w", w=1)
                nc.vector.tensor_tensor(
                    out=pv0, in0=t5w[:, :, :, 0:1], in1=t5w[:, :, :, 1:2], op=OP.add
                )
                # x reflect pads, lane-local (cols 0,25 <- cols 2,23) on the
                # vector engine: it just wrote pooled, so the copies chain
                # with ~zero latency and don't sit behind the ACT FIFO.
                nc.vector.tensor_copy(pv26[:, :, 0:1], pv26[:, :, 2:3])
                nc.vector.tensor_copy(pv26[:, :, 25:26], pv26[:, :, 23:24])

                # ---- build xrep [96=(dy,c), 24y, 26x]: all-contiguous DMAs.
                # Alternate the DMA path per group (SWDGE / HWDGE) so
                # consecutive groups' five-DMA chains run on independent
                # queues instead of serializing on one FIFO.
                dma = nc.gpsimd.dma_start if g % 2 == 0 else nc.scalar.dma_start
                xrep = xrep_pool.tile([96, 24 * 26], BF, tag="xrep")
                # gather quarters straight into the dy=1 block
                dma(out=xrep[32:64, :], in_=pooled[:])
                # dy=0: row slot y holds source row y-1; reflect(-1)=row1
                dma(out=xrep[0:32, 26:624], in_=xrep[32:64, 0:598])
                dma(out=xrep[0:32, 0:26], in_=xrep[32:64, 26:52])
                # dy=2: row slot y holds source row y+1; reflect(24)=row22
                dma(out=xrep[64:96, 0:598], in_=xrep[32:64, 26:624])
                dma(out=xrep[64:96, 598:624], in_=xrep[32:64, 572:598])
                xr3 = xrep[:].rearrange("p (y x) -> p y x", y=OUT, x=26)

                if DEBUG_TAPS:
                    nc.gpsimd.dma_start(
                        out=pooled_out.ap()[g * 32 : (g + 1) * 32],
                        in_=xr3[32:64, :, 1:25],
                    )

                # ---- conv: 2 halves x 3 dx matmuls, K=(dy,ic)=96 ------------
                for half in range(2):
                    pc = psc_pool.tile([32, 288], F32, tag="convps")
                    for dx in range(3):
                        nc.tensor.matmul(
                            pc[:],
                            wt_sb[:, (m * 3 + dx) * 32 : (m * 3 + dx + 1) * 32],
                            xr3[:, 12 * half : 12 * half + 12, dx : dx + 24],
                            start=(dx == 0),
                            stop=(dx == 2),
                        )
                    # LeakyReLU(0.2) on the scalar engine, PSUM -> SBUF bf16
                    nc.scalar.activation(
                        out=v_sb[
                            m * 32 : (m + 1) * 32,
                            t * PIX + half * 288 : t * PIX + (half + 1) * 288,
                        ],
                        in_=pc[:],
                        func=ACT.Prelu,
                        alpha=0.2,
                    )

                # ---- Gram per t once its 3 maps are done --------------------
                if m == 2:
                    if DEBUG_TAPS:
                        nc.gpsimd.dma_start(
                            out=v_out.ap()[:, t * PIX : (t + 1) * PIX],
                            in_=v_sb[:, t * PIX : (t + 1) * PIX],
                        )
                    gp = psg_pool.tile([96, 96], F32, tag="gram")
                    for c5 in range(5):
                        sz = 128 if c5 < 4 else 64
                        vsl = v_sb[:, t * PIX + c5 * 128 : t * PIX + c5 * 128 + sz]
                        pt = pst_pool.tile([128, 96], BF, tag="vtps")
                        nc.tensor.transpose(pt[:sz, :], vsl, id_sb[:])
                        vtt = vt_pool.tile([128, 96], BF, tag="vt")
                        nc.scalar.copy(out=vtt[:sz, :], in_=pt[:sz, :])
                        nc.tensor.matmul(
                            gp[:], vtt[:sz, :], vtt[:sz, :],
                            start=(c5 == 0), stop=(c5 == 4),
                        )
                    nc.scalar.copy(out=g_sb[:, t * 96 : (t + 1) * 96], in_=gp[:])
            # one store for both timesteps' Grams
            nc.sync.dma_start(
                out=g_out.ap().rearrange("t i k -> i t k"), in_=g_sb[:]
            )

    nc.finalize()
    return nc


def _get_nc():
    if "nc" not in _STATE:
        _STATE["nc"] = _build_nc()
    return _STATE["nc"]


def _prep_weights(W1, W2, W3):
    # wt[(dy,ic), (m,dx,oc)] = W_m[oc, ic, dy, dx] / 64   (pool-mean folded in)
    w = np.stack([np.asarray(Wi, np.float64) for Wi in (W1, W2, W3)])
    wt = w.transpose(3, 2, 0, 4, 1).reshape(96, 288) / 64.0
    return wt.astype(ml_dtypes.bfloat16)


def _host_loss(G):
    G = np.asarray(G, np.float64)  # [16, 96, 96]
    T = G.shape[0]
    I96 = np.eye(M)
    Me = I96[None] + ALPHA_E * G
    ld_e = 2.0 * np.log(
        np.diagonal(np.linalg.cholesky(Me), axis1=-2, axis2=-1)
    ).sum()
    blocks = np.stack(
        [G[:, 32 * c : 32 * (c + 1), 32 * c : 32 * (c + 1)] for c in range(3)]
    )  # [3, T, 32, 32]
    Mc = np.eye(32)[None, None] + ALPHA_C * blocks
    ld_c = 2.0 * np.log(
        np.diagonal(np.linalg.cholesky(Mc), axis1=-2, axis2=-1)
    ).sum()
    loss_expd = ld_e / (2.0 * T)
    loss_comp = (32.0 / M) * ld_c / (2.0 * T)
    return np.float32(loss_expd - loss_comp)


def run_device(inputs, **kw):
    """Run the bass kernel; returns (G [16,96,96], BassKernelResults)."""
    from concourse.bass_utils import run_bass_kernel_spmd

    nc = _get_nc()
    wt = _prep_weights(inputs["W1"], inputs["W2"], inputs["W3"])
    ident = np.eye(96, dtype=ml_dtypes.bfloat16)
    ms = np.asarray(inputs["ms_fea"], np.float32)
    pan = np.asarray(inputs["pan_fea"], np.float32)
    alf = np.asarray(inputs["all_fea"], np.float32)
    in_maps = []
    for i in range(NCORES):
        sl = slice(TPC * i, TPC * (i + 1))
        # x[t*3+m] = (ms,pan,alf)[m][t]
        xs = np.stack([ms[sl], pan[sl], alf[sl]], axis=1).reshape(
            NG, CCH, H, W
        )
        in_maps.append(
            {"x": xs.astype(ml_dtypes.float8_e4m3fn), "wt": wt, "ident": ident}
        )
    res = run_bass_kernel_spmd(nc, in_maps, core_ids=list(range(NCORES)), **kw)
    G = np.concatenate([np.asarray(r["g_out"]) for r in res.results], axis=0)
    return G, res


def kernel(**inputs):
    G, _ = run_device(inputs)
    return _host_loss(G)


# revision 21
# speedup vs baseline: 2.2783x; 1.0056x over previous
"""MCR loss kernel for Trainium2 (8 NeuronCores).

Strategy:
  - Shard batch T=16 -> 2 timesteps per core (data parallel, no collectives).
  - Host converts inputs to bf16: halves HBM traffic (the roofline term) and
    enables the DVE 2x packed mode for the pooling adds.
  - Per core, 6 plane-groups (2 timesteps x 3 maps).  Each group's 32x192x192
    plane stack is one contiguous 2.25MB DMA into [128=(c,quarter), 9216].
  - 8x8 avg-pool (as sum; 1/64 folded into conv weights) via a 6-op
    tensor_tensor pairwise tree on the vector engine (2x mode on bf16).
  - Reflect-pad + dy-replication built by small SBUF->SBUF DMAs on the
    scalar-engine HWDGE ring; 3x3 conv as 3 PE matmuls with K=(dy,ic)=96;
    LeakyReLU(0.2) natively on the scalar engine (Lrelu, PSUM->SBUF).
  - Gram G_t = V_t V_t^T via PE transpose + bf16 matmul (f32 PSUM accum).
  - Host: matrix determinant lemma
        logdet(I_576 + a V^T V) = logdet(I_96 + a V V^T)
    so only the [2,96,96] Grams leave the device; float64 Cholesky logdets
    finish the scalar loss.
"""

import numpy as np
import ml_dtypes

_STATE = {}

# -------- fixed problem geometry (hardcoded per harness contract) --------
B, CCH, H, W = 16, 32, 192, 192
NCORES = 8
TPC = B // NCORES          # timesteps per core = 2
NG = TPC * 3               # plane groups per core
OUT = 24                   # pooled spatial size
PIX = OUT * OUT            # 576
M = 96                     # feature rows (3 maps x 32 channels)
ALPHA_E = 6.0              # 576 / (96 * eps)
ALPHA_C = 18.0             # 576 / (32 * eps)

DEBUG_TAPS = False


def _build_nc():
    import concourse.bass as bass
    import concourse.tile as tile
    from concourse import bacc, mybir

    BF = mybir.dt.bfloat16
    F8 = mybir.dt.float8e4
    F32 = mybir.dt.float32
    ACT = mybir.ActivationFunctionType
    OP = mybir.AluOpType

    nc = bacc.Bacc(
        "TRN2", target_bir_lowering=False, debug=False, num_devices=NCORES
    )

    x = nc.declare_dram_parameter("x", [NG, CCH, H, W], F8, isOutput=False)
    wt = nc.declare_dram_parameter("wt", [96, 288], BF, isOutput=False)
    ident = nc.declare_dram_parameter("ident", [96, 96], BF, isOutput=False)
    g_out = nc.declare_dram_parameter("g_out", [TPC, M, M], F32, isOutput=True)
    if DEBUG_TAPS:
        pooled_out = nc.declare_dram_parameter(
            "pooled_out", [NG * 32, PIX], F32, isOutput=True
        )
        v_out = nc.declare_dram_parameter("v_out", [96, TPC * PIX], F32, isOutput=True)

    with tile.TileContext(nc) as tc:
        with (
            tc.tile_pool(name="persist", bufs=1) as persist,
            tc.tile_pool(name="slab", bufs=1) as slab_pool,
            tc.tile_pool(name="red", bufs=3) as red_pool,
            tc.tile_pool(name="xrep", bufs=2) as xrep_pool,
            tc.tile_pool(name="vt", bufs=2) as vt_pool,
            tc.tile_pool(name="psc", bufs=2, space="PSUM") as psc_pool,
            tc.tile_pool(name="pst", bufs=2, space="PSUM") as pst_pool,
            tc.tile_pool(name="psg", bufs=2, space="PSUM") as psg_pool,
        ):
            # wt/id go on the scalar HWDGE ring so slab 0 is first on sync
            wt_sb = persist.tile([96, 288], BF, tag="wt")
            nc.sync.dma_start(out=wt_sb[:], in_=wt.ap())
            id_sb = persist.tile([96, 96], BF, tag="id")
            nc.sync.dma_start(out=id_sb[:], in_=ident.ap())
            v_sb = persist.tile([96, TPC * PIX], BF, tag="v")
            g_sb = persist.tile([96, TPC * 96], F32, tag="g")

            # ---- prefetch all 6 slabs up front: each is one contiguous
            # 2.25MB DMA, partition=(c, quarter).  Dispatching them before
            # any small DMA keeps the 8 shared HWDGE completion-semaphore
            # lanes fresh — interleaving would chain slab loads behind tiny
            # sbuf-to-sbuf DMAs from two groups earlier.
            slabs = []
            for g in range(NG):
                slab = slab_pool.tile([128, 9216], BF, tag=f"slab{g}")
                # SWDGE cast-DMA: reads fp8 from HBM, writes bf16 to SBUF
                nc.gpsimd.dma_start(
                    out=slab[:],
                    in_=x.ap()[g].rearrange("c (q h) w -> (c q) (h w)", q=4),
                )
                slabs.append(slab)

            for g in range(NG):
                t, m = divmod(g, 3)
                slab = slabs[g]

                # ---- pooling: pairwise tensor_tensor tree (bf16 2x mode) ----
                # per partition: 48 rows x 192 cols = (48h, 24x, 8w)
                sv = slab[:].rearrange("p (h x w) -> p h x w", h=48, x=24, w=8)
                t1 = red_pool.tile([128, 4608], BF, tag="t1")
                t1v = t1[:].rearrange("p (h x w) -> p h x w", h=48, x=24, w=4)
                nc.vector.tensor_tensor(
                    out=t1v, in0=sv[:, :, :, 0:4], in1=sv[:, :, :, 4:8], op=OP.add
                )
                t2 = red_pool.tile([128, 2304], BF, tag="t2")
                t2v = t2[:].rearrange("p (h x w) -> p h x w", h=48, x=24, w=2)
                nc.vector.tensor_tensor(
                    out=t2v, in0=t1v[:, :, :, 0:2], in1=t1v[:, :, :, 2:4], op=OP.add
                )
                # h-direction 8:1 before the final w-pair: (6y, 8r, 48xw)
                t2r = t2[:].rearrange("p (y r s) -> p y r s", y=6, r=8, s=48)
                t3 = red_pool.tile([128, 1152], BF, tag="t3")
                t3v = t3[:].rearrange("p (y r s) -> p y r s", y=6, r=4, s=48)
                nc.vector.tensor_tensor(
                    out=t3v, in0=t2r[:, :, 0:4, :], in1=t2r[:, :, 4:8, :], op=OP.add
                )
                t4 = red_pool.tile([128, 576], BF, tag="t4")
                t4v = t4[:].rearrange("p (y r s) -> p y r s", y=6, r=2, s=48)
                nc.vector.tensor_tensor(
                    out=t4v, in0=t3v[:, :, 0:2, :], in1=t3v[:, :, 2:4, :], op=OP.add
                )
                t5 = red_pool.tile([128, 288], BF, tag="t5")
                t5v = t5[:].rearrange("p (y r s) -> p y r s", y=6, r=1, s=48)
                nc.vector.tensor_tensor(
                    out=t5v, in0=t4v[:, :, 0:1, :], in1=t4v[:, :, 1:2, :], op=OP.add
                )
                # final w-pair written straight into the x-padded 26-wide row
                # layout: pooled [128=(c,q), (6y, 26x)], cols 1..24 are data
                t5w = t5[:].rearrange("p (y x w) -> p y x w", y=6, x=24, w=2)
                pooled = red_pool.tile([128, 6 * 26], BF, tag="pooled")
                pv26 = pooled[:].rearrange("p (y x) -> p y x", y=6, x=26)
                pv0 = pv26[:, :, 1:25].rearrange("p y (x w) -> p y x w", w=1)
                nc.vector.tensor_tensor(
                    out=pv0, in0=t5w[:, :, :, 0:1], in1=t5w[:, :, :, 1:2], op=OP.add
                )
                # x reflect pads, lane-local (cols 0,25 <- cols 2,23) on the
                # vector engine: it just wrote pooled, so the copies chain
                # with ~zero latency and don't sit behind the ACT FIFO.
                nc.vector.tensor_copy(pv26[:, :, 0:1], pv26[:, :, 2:3])
                nc.vector.tensor_copy(pv26[:, :, 25:26], pv26[:, :, 23:24])

                # ---- build xrep [96=(dy,c), 24y, 26x] with 7 one-hop DMAs
                # straight from pooled (no second hop -> one completion
                # round-trip of latency), split across the SWDGE and HWDGE
                # paths so they drain in parallel.
                xrep = xrep_pool.tile([96, 24 * 26], BF, tag="xrep")
                pv3 = pooled[:].rearrange("p (y x) -> p y x", x=26)
                xv = xrep[:].rearrange("p (q y x) -> p q y x", q=4, x=26)
                # dy=1: rows 0..23 straight (gather quarters)
                nc.gpsimd.dma_start(out=xrep[32:64, :], in_=pooled[:])
                # dy-shifted interiors, one hop from pooled (full-partition
                # level-1 source APs; multi-level partition sources mislower)
                nc.scalar.dma_start(out=xv[0:32, :, 1:6, :], in_=pv3[:, 0:5, :])
                nc.gpsimd.dma_start(out=xv[64:96, :, 0:5, :], in_=pv3[:, 1:6, :])
                # cross-quarter + reflect rows come off the gathered dy=1
                # block (tiny 78/26-element copies)
                xv1 = xrep[32:64].rearrange("p (q y x) -> p q y x", q=4, x=26)
                nc.scalar.dma_start(
                    out=xv[0:32, 1:4, 0:1, :], in_=xv1[:, 0:3, 5:6, :]
                )
                nc.gpsimd.dma_start(
                    out=xv[0:32, 0:1, 0:1, :], in_=xv1[:, 0:1, 1:2, :]
                )
                nc.scalar.dma_start(
                    out=xv[64:96, 0:3, 5:6, :], in_=xv1[:, 1:4, 0:1, :]
                )
                nc.gpsimd.dma_start(
                    out=xv[64:96, 3:4, 5:6, :], in_=xv1[:, 3:4, 4:5, :]
                )
                xr3 = xrep[:].rearrange("p (y x) -> p y x", y=OUT, x=26)

                if DEBUG_TAPS:
                    nc.gpsimd.dma_start(
                        out=pooled_out.ap()[g * 32 : (g + 1) * 32],
                        in_=xr3[32:64, :, 1:25],
                    )

                # ---- conv: 2 halves x 3 dx matmuls, K=(dy,ic)=96 ------------
                for half in range(2):
                    pc = psc_pool.tile([32, 288], F32, tag="convps")
                    for dx in range(3):
                        nc.tensor.matmul(
                            pc[:],
                            wt_sb[:, (m * 3 + dx) * 32 : (m * 3 + dx + 1) * 32],
                            xr3[:, 12 * half : 12 * half + 12, dx : dx + 24],
                            start=(dx == 0),
                            stop=(dx == 2),
                        )
                    # LeakyReLU(0.2) on the scalar engine, PSUM -> SBUF bf16
                    nc.scalar.activation(
                        out=v_sb[
                            m * 32 : (m + 1) * 32,
                            t * PIX + half * 288 : t * PIX + (half + 1) * 288,
                        ],
                        in_=pc[:],
                        func=ACT.Prelu,
                        alpha=0.2,
                    )

                # ---- Gram per t once its 3 maps are done --------------------
                if m == 2:
                    if DEBUG_TAPS:
                        nc.gpsimd.dma_start(
                            out=v_out.ap()[:, t * PIX : (t + 1) * PIX],
                            in_=v_sb[:, t * PIX : (t + 1) * PIX],
                        )
                    gp = psg_pool.tile([96, 96], F32, tag="gram")
                    for c5 in range(5):
                        sz = 128 if c5 < 4 else 64
                        vsl = v_sb[:, t * PIX + c5 * 128 : t * PIX + c5 * 128 + sz]
                        pt = pst_pool.tile([128, 96], BF, tag="vtps")
                        nc.tensor.transpose(pt[:sz, :], vsl, id_sb[:])
                        vtt = vt_pool.tile([128, 96], BF, tag="vt")
                        nc.scalar.copy(out=vtt[:sz, :], in_=pt[:sz, :])
                        nc.tensor.matmul(
                            gp[:], vtt[:sz, :], vtt[:sz, :],
                            start=(c5 == 0), stop=(c5 == 4),
                        )
                    nc.scalar.copy(out=g_sb[:, t * 96 : (t + 1) * 96], in_=gp[:])
            # one store for both timesteps' Grams
            nc.sync.dma_start(
                out=g_out.ap().rearrange("t i k -> i t k"), in_=g_sb[:]
            )

    nc.finalize()
    return nc


def _get_nc():
    if "nc" not in _STATE:
        _STATE["nc"] = _build_nc()
    return _STATE["nc"]


def _prep_weights(W1, W2, W3):
    # wt[(dy,ic), (m,dx,oc)] = W_m[oc, ic, dy, dx] / 64   (pool-mean folded in)
    w = np.stack([np.asarray(Wi, np.float64) for Wi in (W1, W2, W3)])
    wt = w.transpose(3, 2, 0, 4, 1).reshape(96, 288) / 64.0
    return wt.astype(ml_dtypes.bfloat16)


def _host_loss(G):
    G = np.asarray(G, np.float64)  # [16, 96, 96]
    T = G.shape[0]
    I96 = np.eye(M)
    Me = I96[None] + ALPHA_E * G
    ld_e = 2.0 * np.log(
        np.diagonal(np.linalg.cholesky(Me), axis1=-2, axis2=-1)
    ).sum()
    blocks = np.stack(
        [G[:, 32 * c : 32 * (c + 1), 32 * c : 32 * (c + 1)] for c in range(3)]
    )  # [3, T, 32, 32]
    Mc = np.eye(32)[None, None] + ALPHA_C * blocks
    ld_c = 2.0 * np.log(
        np.diagonal(np.linalg.cholesky(Mc), axis1=-2, axis2=-1)
    ).sum()
    loss_expd = ld_e / (2.0 * T)
    loss_comp = (32.0 / M) * ld_c / (2.0 * T)
    return np.float32(loss_expd - loss_comp)


def run_device(inputs, **kw):
    """Run the bass kernel; returns (G [16,96,96], BassKernelResults)."""
    from concourse.bass_utils import run_bass_kernel_spmd

    nc = _get_nc()
    wt = _prep_weights(inputs["W1"], inputs["W2"], inputs["W3"])
    ident = np.eye(96, dtype=ml_dtypes.bfloat16)
    ms = np.asarray(inputs["ms_fea"], np.float32)
    pan = np.asarray(inputs["pan_fea"], np.float32)
    alf = np.asarray(inputs["all_fea"], np.float32)
    in_maps = []
    for i in range(NCORES):
        sl = slice(TPC * i, TPC * (i + 1))
        # x[t*3+m] = (ms,pan,alf)[m][t]
        xs = np.stack([ms[sl], pan[sl], alf[sl]], axis=1).reshape(
            NG, CCH, H, W
        )
        in_maps.append(
            {"x": xs.astype(ml_dtypes.float8_e4m3fn), "wt": wt, "ident": ident}
        )
    res = run_bass_kernel_spmd(nc, in_maps, core_ids=list(range(NCORES)), **kw)
    G = np.concatenate([np.asarray(r["g_out"]) for r in res.results], axis=0)
    return G, res


def kernel(**inputs):
    G, _ = run_device(inputs)
    return _host_loss(G)


# revision 26
# speedup vs baseline: 2.7404x; 1.2028x over previous
"""MCR loss kernel for Trainium2 (8 NeuronCores).

Strategy:
  - Shard batch T=16 -> 2 timesteps per core (data parallel, no collectives).
  - Host converts inputs to bf16: halves HBM traffic (the roofline term) and
    enables the DVE 2x packed mode for the pooling adds.
  - Per core, 6 plane-groups (2 timesteps x 3 maps).  Each group's 32x192x192
    plane stack is one contiguous 2.25MB DMA into [128=(c,quarter), 9216].
  - 8x8 avg-pool (as sum; 1/64 folded into conv weights) via a 6-op
    tensor_tensor pairwise tree on the vector engine (2x mode on bf16).
  - Reflect-pad + dy-replication built by small SBUF->SBUF DMAs on the
    scalar-engine HWDGE ring; 3x3 conv as 3 PE matmuls with K=(dy,ic)=96;
    LeakyReLU(0.2) natively on the scalar engine (Lrelu, PSUM->SBUF).
  - Gram G_t = V_t V_t^T via PE transpose + bf16 matmul (f32 PSUM accum).
  - Host: matrix determinant lemma
        logdet(I_576 + a V^T V) = logdet(I_96 + a V V^T)
    so only the [2,96,96] Grams leave the device; float64 Cholesky logdets
    finish the scalar loss.
"""

import numpy as np
import ml_dtypes

_STATE = {}

# -------- fixed problem geometry (hardcoded per harness contract) --------
B, CCH, H, W = 16, 32, 192, 192
NCORES = 8
TPC = B // NCORES          # timesteps per core = 2
NG = TPC * 3               # plane groups per core
OUT = 24                   # pooled spatial size
PIX = OUT * OUT            # 576
M = 96                     # feature rows (3 maps x 32 channels)
ALPHA_E = 6.0              # 576 / (96 * eps)
ALPHA_C = 18.0             # 576 / (32 * eps)

DEBUG_TAPS = False


def _build_nc():
    import concourse.bass as bass
    import concourse.tile as tile
    from concourse import bacc, mybir

    BF = mybir.dt.bfloat16
    F8 = mybir.dt.float8e4
    F32 = mybir.dt.float32
    ACT = mybir.ActivationFunctionType
    OP = mybir.AluOpType

    nc = bacc.Bacc(
        "TRN2", target_bir_lowering=False, debug=False, num_devices=NCORES
    )

    x = nc.declare_dram_parameter("x", [NG, CCH, H, W], F8, isOutput=False)
    wt = nc.declare_dram_parameter("wt", [96, 288], BF, isOutput=False)
    ident = nc.declare_dram_parameter("ident", [96, 96], BF, isOutput=False)
    # sel[:, q*32+c] = e_{c*4+q}: column-permuted identity; sel[:, q*32:...]
    # as matmul lhsT gathers partition (c,q) -> output row c
    sel = nc.declare_dram_parameter("sel", [128, 128], BF, isOutput=False)
    g_out = nc.declare_dram_parameter("g_out", [TPC, M, M], F32, isOutput=True)
    if DEBUG_TAPS:
        pooled_out = nc.declare_dram_parameter(
            "pooled_out", [NG * 32, PIX], F32, isOutput=True
        )
        v_out = nc.declare_dram_parameter("v_out", [96, TPC * PIX], F32, isOutput=True)

    with tile.TileContext(nc) as tc:
        with (
            tc.tile_pool(name="persist", bufs=1) as persist,
            tc.tile_pool(name="slab", bufs=1) as slab_pool,
            tc.tile_pool(name="red", bufs=3) as red_pool,
            tc.tile_pool(name="xrep", bufs=2) as xrep_pool,
            tc.tile_pool(name="vt", bufs=2) as vt_pool,
            tc.tile_pool(name="psc", bufs=2, space="PSUM") as psc_pool,
            tc.tile_pool(name="pst", bufs=2, space="PSUM") as pst_pool,
            tc.tile_pool(name="psg", bufs=1, space="PSUM") as psg_pool,
            tc.tile_pool(name="psx", bufs=1, space="PSUM") as psx_pool,
        ):
            wt_sb = persist.tile([96, 288], BF, tag="wt")
            nc.sync.dma_start(out=wt_sb[:], in_=wt.ap())
            id_sb = persist.tile([96, 96], BF, tag="id")
            nc.sync.dma_start(out=id_sb[:], in_=ident.ap())
            sel_sb = persist.tile([128, 128], BF, tag="sel")
            nc.sync.dma_start(out=sel_sb[:], in_=sel.ap())
            v_sb = persist.tile([96, TPC * PIX], BF, tag="v")
            g_sb = persist.tile([96, TPC * 96], F32, tag="g")

            # ---- prefetch all 6 slabs up front: each is one contiguous
            # 2.25MB DMA, partition=(c, quarter).  Dispatching them before
            # any small DMA keeps the 8 shared HWDGE completion-semaphore
            # lanes fresh — interleaving would chain slab loads behind tiny
            # sbuf-to-sbuf DMAs from two groups earlier.
            slabs = []
            for g in range(NG):
                slab = slab_pool.tile([128, 9216], BF, tag=f"slab{g}")
                # SWDGE cast-DMA: reads fp8 from HBM, writes bf16 to SBUF
                nc.gpsimd.dma_start(
                    out=slab[:],
                    in_=x.ap()[g].rearrange("c (q h) w -> (c q) (h w)", q=4),
                )
                slabs.append(slab)

            for g in range(NG):
                t, m = divmod(g, 3)
                slab = slabs[g]

                # ---- pooling: pairwise tensor_tensor tree (bf16 2x mode) ----
                # per partition: 48 rows x 192 cols = (48h, 24x, 8w)
                sv = slab[:].rearrange("p (h x w) -> p h x w", h=48, x=24, w=8)
                t1 = red_pool.tile([128, 4608], BF, tag="t1")
                t1v = t1[:].rearrange("p (h x w) -> p h x w", h=48, x=24, w=4)
                nc.vector.tensor_tensor(
                    out=t1v, in0=sv[:, :, :, 0:4], in1=sv[:, :, :, 4:8], op=OP.add
                )
                t2 = red_pool.tile([128, 2304], BF, tag="t2")
                t2v = t2[:].rearrange("p (h x w) -> p h x w", h=48, x=24, w=2)
                nc.vector.tensor_tensor(
                    out=t2v, in0=t1v[:, :, :, 0:2], in1=t1v[:, :, :, 2:4], op=OP.add
                )
                # h-direction 8:1 before the final w-pair: (6y, 8r, 48xw)
                t2r = t2[:].rearrange("p (y r s) -> p y r s", y=6, r=8, s=48)
                t3 = red_pool.tile([128, 1152], BF, tag="t3")
                t3v = t3[:].rearrange("p (y r s) -> p y r s", y=6, r=4, s=48)
                nc.vector.tensor_tensor(
                    out=t3v, in0=t2r[:, :, 0:4, :], in1=t2r[:, :, 4:8, :], op=OP.add
                )
                t4 = red_pool.tile([128, 576], BF, tag="t4")
                t4v = t4[:].rearrange("p (y r s) -> p y r s", y=6, r=2, s=48)
                nc.vector.tensor_tensor(
                    out=t4v, in0=t3v[:, :, 0:2, :], in1=t3v[:, :, 2:4, :], op=OP.add
                )
                t5 = red_pool.tile([128, 288], BF, tag="t5")
                t5v = t5[:].rearrange("p (y r s) -> p y r s", y=6, r=1, s=48)
                nc.vector.tensor_tensor(
                    out=t5v, in0=t4v[:, :, 0:1, :], in1=t4v[:, :, 1:2, :], op=OP.add
                )
                # final w-pair written straight into the x-padded 26-wide row
                # layout: pooled [128=(c,q), (6y, 26x)], cols 1..24 are data
                t5w = t5[:].rearrange("p (y x w) -> p y x w", y=6, x=24, w=2)
                pooled = red_pool.tile([128, 6 * 26], BF, tag="pooled")
                pv26 = pooled[:].rearrange("p (y x) -> p y x", y=6, x=26)
                pv0 = pv26[:, :, 1:25].rearrange("p y (x w) -> p y x w", w=1)
                nc.vector.tensor_tensor(
                    out=pv0, in0=t5w[:, :, :, 0:1], in1=t5w[:, :, :, 1:2], op=OP.add
                )
                # x reflect pads, lane-local (cols 0,25 <- cols 2,23) on the
                # vector engine: it just wrote pooled, so the copies chain
                # with ~zero latency and don't sit behind the ACT FIFO.
                nc.vector.tensor_copy(pv26[:, :, 0:1], pv26[:, :, 2:3])
                nc.vector.tensor_copy(pv26[:, :, 25:26], pv26[:, :, 23:24])

                # ---- build xrep [96=(dy,c), 24y, 26x] on the TENSOR engine.
                # Selector matmuls (lhsT = column-permuted identity slice)
                # remap partitions (c,q) -> (dy,c) and place dy-shifted,
                # reflect-padded row windows — no DMA involved, so the conv
                # never queues behind the HBM slab stream.  Two PSUM tiles
                # hold element halves 0:312 / 312:624 of the 624-el rows.
                psA = psx_pool.tile([96, 312], F32, tag="xpsA")
                psB = psx_pool.tile([96, 312], F32, tag="xpsB")
                # (q, dy-block, tile, dest el0:el1, src el0:el1); dy block d
                # writes row r from source row r + d - 1 (reflect at edges)
                pieces = [
                    (0, 0, psA, 0, 26, 26, 52),      # d0 row0 <- reflect row1
                    (0, 0, psA, 26, 182, 0, 156),    # d0 rows1-6 <- q0 rows0-5
                    (0, 1, psA, 0, 156, 0, 156),     # d1 rows0-5
                    (0, 2, psA, 0, 130, 26, 156),    # d2 rows0-4 <- q0 rows1-5
                    (1, 0, psA, 182, 312, 0, 130),   # d0 rows7-11
                    (1, 0, psB, 0, 26, 130, 156),    # d0 row12
                    (1, 1, psA, 156, 312, 0, 156),   # d1 rows6-11
                    (1, 2, psA, 130, 286, 0, 156),   # d2 rows5-10
                    (2, 0, psB, 26, 182, 0, 156),    # d0 rows13-18
                    (2, 1, psB, 0, 156, 0, 156),     # d1 rows12-17
                    (2, 2, psA, 286, 312, 0, 26),    # d2 row11
                    (2, 2, psB, 0, 130, 26, 156),    # d2 rows12-16
                    (3, 0, psB, 182, 312, 0, 130),   # d0 rows19-23
                    (3, 1, psB, 156, 312, 0, 156),   # d1 rows18-23
                    (3, 2, psB, 130, 286, 0, 156),   # d2 rows17-22
                    (3, 2, psB, 286, 312, 104, 130), # d2 row23 <- reflect row22
                ]
                for q, dblk, psX, e0, e1, s0, s1 in pieces:
                    nc.tensor.matmul(
                        psX[dblk * 32 : (dblk + 1) * 32, e0:e1],
                        sel_sb[:, q * 32 : (q + 1) * 32],
                        pooled[:, s0:s1],
                        start=True,
                        stop=True,
                    )
                xrep = xrep_pool.tile([96, 24 * 26], BF, tag="xrep")
                nc.scalar.copy(out=xrep[:, 0:312], in_=psA[:])
                nc.scalar.copy(out=xrep[:, 312:624], in_=psB[:])
                xr3 = xrep[:].rearrange("p (y x) -> p y x", y=OUT, x=26)

                if DEBUG_TAPS:
                    nc.gpsimd.dma_start(
                        out=pooled_out.ap()[g * 32 : (g + 1) * 32],
                        in_=xr3[32:64, :, 1:25],
                    )

                # ---- conv: 2 halves x 3 dx matmuls, K=(dy,ic)=96 ------------
                for half in range(2):
                    pc = psc_pool.tile([32, 288], F32, tag="convps")
                    for dx in range(3):
                        nc.tensor.matmul(
                            pc[:],
                            wt_sb[:, (m * 3 + dx) * 32 : (m * 3 + dx + 1) * 32],
                            xr3[:, 12 * half : 12 * half + 12, dx : dx + 24],
                            start=(dx == 0),
                            stop=(dx == 2),
                        )
                    # LeakyReLU(0.2) on the scalar engine, PSUM -> SBUF bf16
                    nc.scalar.activation(
                        out=v_sb[
                            m * 32 : (m + 1) * 32,
                            t * PIX + half * 288 : t * PIX + (half + 1) * 288,
                        ],
                        in_=pc[:],
                        func=ACT.Prelu,
                        alpha=0.2,
                    )

                # ---- Gram per t once its 3 maps are done --------------------
                if m == 2:
                    if DEBUG_TAPS:
                        nc.gpsimd.dma_start(
                            out=v_out.ap()[:, t * PIX : (t + 1) * PIX],
                            in_=v_sb[:, t * PIX : (t + 1) * PIX],
                        )
                    gp = psg_pool.tile([96, 96], F32, tag="gram")
                    for c5 in range(5):
                        sz = 128 if c5 < 4 else 64
                        vsl = v_sb[:, t * PIX + c5 * 128 : t * PIX + c5 * 128 + sz]
                        pt = pst_pool.tile([128, 96], BF, tag="vtps")
                        nc.tensor.transpose(pt[:sz, :], vsl, id_sb[:])
                        vtt = vt_pool.tile([128, 96], BF, tag="vt")
                        nc.scalar.copy(out=vtt[:sz, :], in_=pt[:sz, :])
                        nc.tensor.matmul(
                            gp[:], vtt[:sz, :], vtt[:sz, :],
                            start=(c5 == 0), stop=(c5 == 4),
                        )
                    nc.scalar.copy(out=g_sb[:, t * 96 : (t + 1) * 96], in_=gp[:])
            # one store for both timesteps' Grams
            nc.sync.dma_start(
                out=g_out.ap().rearrange("t i k -> i t k"), in_=g_sb[:]
            )

    nc.finalize()
    return nc


def _get_nc():
    if "nc" not in _STATE:
        _STATE["nc"] = _build_nc()
    return _STATE["nc"]


def _prep_weights(W1, W2, W3):
    # wt[(dy,ic), (m,dx,oc)] = W_m[oc, ic, dy, dx] / 64   (pool-mean folded in)
    w = np.stack([np.asarray(Wi, np.float64) for Wi in (W1, W2, W3)])
    wt = w.transpose(3, 2, 0, 4, 1).reshape(96, 288) / 64.0
    return wt.astype(ml_dtypes.bfloat16)


def _host_loss(G):
    G = np.asarray(G, np.float64)  # [16, 96, 96]
    T = G.shape[0]
    I96 = np.eye(M)
    Me = I96[None] + ALPHA_E * G
    ld_e = 2.0 * np.log(
        np.diagonal(np.linalg.cholesky(Me), axis1=-2, axis2=-1)
    ).sum()
    blocks = np.stack(
        [G[:, 32 * c : 32 * (c + 1), 32 * c : 32 * (c + 1)] for c in range(3)]
    )  # [3, T, 32, 32]
    Mc = np.eye(32)[None, None] + ALPHA_C * blocks
    ld_c = 2.0 * np.log(
        np.diagonal(np.linalg.cholesky(Mc), axis1=-2, axis2=-1)
    ).sum()
    loss_expd = ld_e / (2.0 * T)
    loss_comp = (32.0 / M) * ld_c / (2.0 * T)
    return np.float32(loss_expd - loss_comp)


def run_device(inputs, **kw):
    """Run the bass kernel; returns (G [16,96,96], BassKernelResults)."""
    from concourse.bass_utils import run_bass_kernel_spmd

    nc = _get_nc()
    wt = _prep_weights(inputs["W1"], inputs["W2"], inputs["W3"])
    ident = np.eye(96, dtype=ml_dtypes.bfloat16)
    # sel[:, q*32+c] = e_{c*4+q}
    perm = np.arange(128).reshape(4, 32).T.reshape(-1) * 0
    perm = np.array([(i % 32) * 4 + (i // 32) for i in range(128)])
    selm = np.eye(128)[:, perm].astype(ml_dtypes.bfloat16)
    ms = np.asarray(inputs["ms_fea"], np.float32)
    pan = np.asarray(inputs["pan_fea"], np.float32)
    alf = np.asarray(inputs["all_fea"], np.float32)
    in_maps = []
    for i in range(NCORES):
        sl = slice(TPC * i, TPC * (i + 1))
        # x[t*3+m] = (ms,pan,alf)[m][t]
        xs = np.stack([ms[sl], pan[sl], alf[sl]], axis=1).reshape(
            NG, CCH, H, W
        )
        in_maps.append(
            {"x": xs.astype(ml_dtypes.float8_e4m3fn), "wt": wt, "ident": ident,
             "sel": selm}
        )
    res = run_bass_kernel_spmd(nc, in_maps, core_ids=list(range(NCORES)), **kw)
    G = np.concatenate([np.asarray(r["g_out"]) for r in res.results], axis=0)
    return G, res


def kernel(**inputs):
    G, _ = run_device(inputs)
    return _host_loss(G)


# revision 28
# speedup vs baseline: 2.7846x; 1.0161x over previous
"""MCR loss kernel for Trainium2 (8 NeuronCores).

Strategy:
  - Shard batch T=16 -> 2 timesteps per core (data parallel, no collectives).
  - Host converts inputs to bf16: halves HBM traffic (the roofline term) and
    enables the DVE 2x packed mode for the pooling adds.
  - Per core, 6 plane-groups (2 timesteps x 3 maps).  Each group's 32x192x192
    plane stack is one contiguous 2.25MB DMA into [128=(c,quarter), 9216].
  - 8x8 avg-pool (as sum; 1/64 folded into conv weights) via a 6-op
    tensor_tensor pairwise tree on the vector engine (2x mode on bf16).
  - Reflect-pad + dy-replication built by small SBUF->SBUF DMAs on the
    scalar-engine HWDGE ring; 3x3 conv as 3 PE matmuls with K=(dy,ic)=96;
    LeakyReLU(0.2) natively on the scalar engine (Lrelu, PSUM->SBUF).
  - Gram G_t = V_t V_t^T via PE transpose + bf16 matmul (f32 PSUM accum).
  - Host: matrix determinant lemma
        logdet(I_576 + a V^T V) = logdet(I_96 + a V V^T)
    so only the [2,96,96] Grams leave the device; float64 Cholesky logdets
    finish the scalar loss.
"""

import numpy as np
import ml_dtypes

_STATE = {}

# -------- fixed problem geometry (hardcoded per harness contract) --------
B, CCH, H, W = 16, 32, 192, 192
NCORES = 8
TPC = B // NCORES          # timesteps per core = 2
NG = TPC * 3               # plane groups per core
OUT = 24                   # pooled spatial size
PIX = OUT * OUT            # 576
M = 96                     # feature rows (3 maps x 32 channels)
ALPHA_E = 6.0              # 576 / (96 * eps)
ALPHA_C = 18.0             # 576 / (32 * eps)

DEBUG_TAPS = False


def _build_nc():
    import concourse.bass as bass
    import concourse.tile as tile
    from concourse import bacc, mybir

    BF = mybir.dt.bfloat16
    F8 = mybir.dt.float8e4
    F32 = mybir.dt.float32
    ACT = mybir.ActivationFunctionType
    OP = mybir.AluOpType

    nc = bacc.Bacc(
        "TRN2", target_bir_lowering=False, debug=False, num_devices=NCORES
    )

    x = nc.declare_dram_parameter("x", [NG, CCH, H, W], F8, isOutput=False)
    wt = nc.declare_dram_parameter("wt", [96, 288], BF, isOutput=False)
    ident = nc.declare_dram_parameter("ident", [96, 96], BF, isOutput=False)
    # sel[:, q*32+c] = e_{c*4+q}: column-permuted identity; sel[:, q*32:...]
    # as matmul lhsT gathers partition (c,q) -> output row c
    sel = nc.declare_dram_parameter("sel", [128, 128], BF, isOutput=False)
    g_out = nc.declare_dram_parameter("g_out", [TPC, M, M], F32, isOutput=True)
    if DEBUG_TAPS:
        pooled_out = nc.declare_dram_parameter(
            "pooled_out", [NG * 32, PIX], F32, isOutput=True
        )
        v_out = nc.declare_dram_parameter("v_out", [96, TPC * PIX], F32, isOutput=True)

    with tile.TileContext(nc) as tc:
        with (
            tc.tile_pool(name="persist", bufs=1) as persist,
            tc.tile_pool(name="slab", bufs=1) as slab_pool,
            tc.tile_pool(name="red", bufs=3) as red_pool,
            tc.tile_pool(name="xrep", bufs=2) as xrep_pool,
            tc.tile_pool(name="vt", bufs=2) as vt_pool,
            tc.tile_pool(name="psc", bufs=2, space="PSUM") as psc_pool,
            tc.tile_pool(name="pst", bufs=2, space="PSUM") as pst_pool,
            tc.tile_pool(name="psg", bufs=1, space="PSUM") as psg_pool,
            tc.tile_pool(name="psx", bufs=1, space="PSUM") as psx_pool,
        ):
            wt_sb = persist.tile([96, 288], BF, tag="wt")
            nc.sync.dma_start(out=wt_sb[:], in_=wt.ap())
            id_sb = persist.tile([96, 96], BF, tag="id")
            nc.sync.dma_start(out=id_sb[:], in_=ident.ap())
            sel_sb = persist.tile([128, 128], BF, tag="sel")
            nc.sync.dma_start(out=sel_sb[:], in_=sel.ap())
            v_sb = persist.tile([96, TPC * PIX], BF, tag="v")
            g_sb = persist.tile([96, TPC * 96], F32, tag="g")

            # ---- prefetch all 6 slabs up front: each is one contiguous
            # 2.25MB DMA, partition=(c, quarter).  Dispatching them before
            # any small DMA keeps the 8 shared HWDGE completion-semaphore
            # lanes fresh — interleaving would chain slab loads behind tiny
            # sbuf-to-sbuf DMAs from two groups earlier.
            slabs = []
            for g in range(NG):
                slab = slab_pool.tile([128, 9216], BF, tag=f"slab{g}")
                # SWDGE cast-DMA: reads fp8 from HBM, writes bf16 to SBUF.
                # Two half-slab DMAs per group for finer pipelining: the
                # first L1 starts half a slab earlier, and the tail begins
                # half a slab sooner.
                xg = x.ap()[g].rearrange("c (q h) w -> (c q) (h w)", q=4)
                nc.gpsimd.dma_start(out=slab[:, 0:4608], in_=xg[:, 0:4608])
                nc.gpsimd.dma_start(out=slab[:, 4608:9216], in_=xg[:, 4608:9216])
                slabs.append(slab)

            for g in range(NG):
                t, m = divmod(g, 3)
                slab = slabs[g]

                # ---- pooling: pairwise tensor_tensor tree (bf16 2x mode) ----
                # per partition: 48 rows x 192 cols = (48h, 24x, 8w).
                # L1 split into row-halves matching the two half-slab DMAs.
                sv = slab[:].rearrange("p (h x w) -> p h x w", h=48, x=24, w=8)
                t1 = red_pool.tile([128, 4608], BF, tag="t1")
                t1v = t1[:].rearrange("p (h x w) -> p h x w", h=48, x=24, w=4)
                nc.vector.tensor_tensor(
                    out=t1v[:, 0:24],
                    in0=sv[:, 0:24, :, 0:4],
                    in1=sv[:, 0:24, :, 4:8],
                    op=OP.add,
                )
                nc.vector.tensor_tensor(
                    out=t1v[:, 24:48],
                    in0=sv[:, 24:48, :, 0:4],
                    in1=sv[:, 24:48, :, 4:8],
                    op=OP.add,
                )
                t2 = red_pool.tile([128, 2304], BF, tag="t2")
                t2v = t2[:].rearrange("p (h x w) -> p h x w", h=48, x=24, w=2)
                nc.vector.tensor_tensor(
                    out=t2v, in0=t1v[:, :, :, 0:2], in1=t1v[:, :, :, 2:4], op=OP.add
                )
                # h-direction 8:1 before the final w-pair: (6y, 8r, 48xw)
                t2r = t2[:].rearrange("p (y r s) -> p y r s", y=6, r=8, s=48)
                t3 = red_pool.tile([128, 1152], BF, tag="t3")
                t3v = t3[:].rearrange("p (y r s) -> p y r s", y=6, r=4, s=48)
                nc.vector.tensor_tensor(
                    out=t3v, in0=t2r[:, :, 0:4, :], in1=t2r[:, :, 4:8, :], op=OP.add
                )
                t4 = red_pool.tile([128, 576], BF, tag="t4")
                t4v = t4[:].rearrange("p (y r s) -> p y r s", y=6, r=2, s=48)
                nc.vector.tensor_tensor(
                    out=t4v, in0=t3v[:, :, 0:2, :], in1=t3v[:, :, 2:4, :], op=OP.add
                )
                t5 = red_pool.tile([128, 288], BF, tag="t5")
                t5v = t5[:].rearrange("p (y r s) -> p y r s", y=6, r=1, s=48)
                nc.vector.tensor_tensor(
                    out=t5v, in0=t4v[:, :, 0:1, :], in1=t4v[:, :, 1:2, :], op=OP.add
                )
                # final w-pair written straight into the x-padded 26-wide row
                # layout: pooled [128=(c,q), (6y, 26x)], cols 1..24 are data
                t5w = t5[:].rearrange("p (y x w) -> p y x w", y=6, x=24, w=2)
                pooled = red_pool.tile([128, 6 * 26], BF, tag="pooled")
                pv26 = pooled[:].rearrange("p (y x) -> p y x", y=6, x=26)
                pv0 = pv26[:, :, 1:25].rearrange("p y (x w) -> p y x w", w=1)
                nc.vector.tensor_tensor(
                    out=pv0, in0=t5w[:, :, :, 0:1], in1=t5w[:, :, :, 1:2], op=OP.add
                )
                # x reflect pads, lane-local (cols 0,25 <- cols 2,23) on the
                # vector engine: it just wrote pooled, so the copies chain
                # with ~zero latency and don't sit behind the ACT FIFO.
                nc.vector.tensor_copy(pv26[:, :, 0:1], pv26[:, :, 2:3])
                nc.vector.tensor_copy(pv26[:, :, 25:26], pv26[:, :, 23:24])

                # ---- build xrep [96=(dy,c), 24y, 26x] on the TENSOR engine.
                # Selector matmuls (lhsT = column-permuted identity slice)
                # remap partitions (c,q) -> (dy,c) and place dy-shifted,
                # reflect-padded row windows — no DMA involved, so the conv
                # never queues behind the HBM slab stream.  Two PSUM tiles
                # hold element halves 0:312 / 312:624 of the 624-el rows.
                psA = psx_pool.tile([96, 312], F32, tag="xpsA")
                psB = psx_pool.tile([96, 312], F32, tag="xpsB")
                # (q, dy-block, tile, dest el0:el1, src el0:el1); dy block d
                # writes row r from source row r + d - 1 (reflect at edges)
                pieces = [
                    (0, 0, psA, 0, 26, 26, 52),      # d0 row0 <- reflect row1
                    (0, 0, psA, 26, 182, 0, 156),    # d0 rows1-6 <- q0 rows0-5
                    (0, 1, psA, 0, 156, 0, 156),     # d1 rows0-5
                    (0, 2, psA, 0, 130, 26, 156),    # d2 rows0-4 <- q0 rows1-5
                    (1, 0, psA, 182, 312, 0, 130),   # d0 rows7-11
                    (1, 0, psB, 0, 26, 130, 156),    # d0 row12
                    (1, 1, psA, 156, 312, 0, 156),   # d1 rows6-11
                    (1, 2, psA, 130, 286, 0, 156),   # d2 rows5-10
                    (2, 0, psB, 26, 182, 0, 156),    # d0 rows13-18
                    (2, 1, psB, 0, 156, 0, 156),     # d1 rows12-17
                    (2, 2, psA, 286, 312, 0, 26),    # d2 row11
                    (2, 2, psB, 0, 130, 26, 156),    # d2 rows12-16
                    (3, 0, psB, 182, 312, 0, 130),   # d0 rows19-23
                    (3, 1, psB, 156, 312, 0, 156),   # d1 rows18-23
                    (3, 2, psB, 130, 286, 0, 156),   # d2 rows17-22
                    (3, 2, psB, 286, 312, 104, 130), # d2 row23 <- reflect row22
                ]
                for q, dblk, psX, e0, e1, s0, s1 in pieces:
                    nc.tensor.matmul(
                        psX[dblk * 32 : (dblk + 1) * 32, e0:e1],
                        sel_sb[:, q * 32 : (q + 1) * 32],
                        pooled[:, s0:s1],
                        start=True,
                        stop=True,
                    )
                xrep = xrep_pool.tile([96, 24 * 26], BF, tag="xrep")
                nc.scalar.copy(out=xrep[:, 0:312], in_=psA[:])
                nc.scalar.copy(out=xrep[:, 312:624], in_=psB[:])
                xr3 = xrep[:].rearrange("p (y x) -> p y x", y=OUT, x=26)

                if DEBUG_TAPS:
                    nc.gpsimd.dma_start(
                        out=pooled_out.ap()[g * 32 : (g + 1) * 32],
                        in_=xr3[32:64, :, 1:25],
                    )

                # ---- conv: 2 halves x 3 dx matmuls, K=(dy,ic)=96 ------------
                for half in range(2):
                    pc = psc_pool.tile([32, 288], F32, tag="convps")
                    for dx in range(3):
                        nc.tensor.matmul(
                            pc[:],
                            wt_sb[:, (m * 3 + dx) * 32 : (m * 3 + dx + 1) * 32],
                            xr3[:, 12 * half : 12 * half + 12, dx : dx + 24],
                            start=(dx == 0),
                            stop=(dx == 2),
                        )
                    # LeakyReLU(0.2) on the scalar engine, PSUM -> SBUF bf16
                    nc.scalar.activation(
                        out=v_sb[
                            m * 32 : (m + 1) * 32,
                            t * PIX + half * 288 : t * PIX + (half + 1) * 288,
                        ],
                        in_=pc[:],
                        func=ACT.Prelu,
                        alpha=0.2,
                    )

                # ---- Gram per t once its 3 maps are done --------------------
                if m == 2:
                    if DEBUG_TAPS:
                        nc.gpsimd.dma_start(
                            out=v_out.ap()[:, t * PIX : (t + 1) * PIX],
                            in_=v_sb[:, t * PIX : (t + 1) * PIX],
                        )
                    gp = psg_pool.tile([96, 96], F32, tag="gram")
                    for c5 in range(5):
                        sz = 128 if c5 < 4 else 64
                        vsl = v_sb[:, t * PIX + c5 * 128 : t * PIX + c5 * 128 + sz]
                        pt = pst_pool.tile([128, 96], BF, tag="vtps")
                        nc.tensor.transpose(pt[:sz, :], vsl, id_sb[:])
                        vtt = vt_pool.tile([128, 96], BF, tag="vt")
                        nc.scalar.copy(out=vtt[:sz, :], in_=pt[:sz, :])
                        nc.tensor.matmul(
                            gp[:], vtt[:sz, :], vtt[:sz, :],
                            start=(c5 == 0), stop=(c5 == 4),
                        )
                    nc.scalar.copy(out=g_sb[:, t * 96 : (t + 1) * 96], in_=gp[:])
            # one store for both timesteps' Grams
            nc.sync.dma_start(
                out=g_out.ap().rearrange("t i k -> i t k"), in_=g_sb[:]
            )

    nc.finalize()
    return nc


def _get_nc():
    if "nc" not in _STATE:
        _STATE["nc"] = _build_nc()
    return _STATE["nc"]


def _prep_weights(W1, W2, W3):
    # wt[(dy,ic), (m,dx,oc)] = W_m[oc, ic, dy, dx] / 64   (pool-mean folded in)
    w = np.stack([np.asarray(Wi, np.float64) for Wi in (W1, W2, W3)])
    wt = w.transpose(3, 2, 0, 4, 1).reshape(96, 288) / 64.0
    return wt.astype(ml_dtypes.bfloat16)


def _host_loss(G):
    G = np.asarray(G, np.float64)  # [16, 96, 96]
    T = G.shape[0]
    I96 = np.eye(M)
    Me = I96[None] + ALPHA_E * G
    ld_e = 2.0 * np.log(
        np.diagonal(np.linalg.cholesky(Me), axis1=-2, axis2=-1)
    ).sum()
    blocks = np.stack(
        [G[:, 32 * c : 32 * (c + 1), 32 * c : 32 * (c + 1)] for c in range(3)]
    )  # [3, T, 32, 32]
    Mc = np.eye(32)[None, None] + ALPHA_C * blocks
    ld_c = 2.0 * np.log(
        np.diagonal(np.linalg.cholesky(Mc), axis1=-2, axis2=-1)
    ).sum()
    loss_expd = ld_e / (2.0 * T)
    loss_comp = (32.0 / M) * ld_c / (2.0 * T)
    return np.float32(loss_expd - loss_comp)


def run_device(inputs, **kw):
    """Run the bass kernel; returns (G [16,96,96], BassKernelResults)."""
    from concourse.bass_utils import run_bass_kernel_spmd

    nc = _get_nc()
    wt = _prep_weights(inputs["W1"], inputs["W2"], inputs["W3"])
    ident = np.eye(96, dtype=ml_dtypes.bfloat16)
    # sel[:, q*32+c] = e_{c*4+q}
    perm = np.arange(128).reshape(4, 32).T.reshape(-1) * 0
    perm = np.array([(i % 32) * 4 + (i // 32) for i in range(128)])
    selm = np.eye(128)[:, perm].astype(ml_dtypes.bfloat16)
    ms = np.asarray(inputs["ms_fea"], np.float32)
    pan = np.asarray(inputs["pan_fea"], np.float32)
    alf = np.asarray(inputs["all_fea"], np.float32)
    in_maps = []
    for i in range(NCORES):
        sl = slice(TPC * i, TPC * (i + 1))
        # x[t*3+m] = (ms,pan,alf)[m][t]
        xs = np.stack([ms[sl], pan[sl], alf[sl]], axis=1).reshape(
            NG, CCH, H, W
        )
        in_maps.append(
            {"x": xs.astype(ml_dtypes.float8_e4m3fn), "wt": wt, "ident": ident,
             "sel": selm}
        )
    res = run_bass_kernel_spmd(nc, in_maps, core_ids=list(range(NCORES)), **kw)
    G = np.concatenate([np.asarray(r["g_out"]) for r in res.results], axis=0)
    return G, res


def kernel(**inputs):
    G, _ = run_device(inputs)
    return _host_loss(G)
